# revision 6
# baseline (speedup 1.0000x reference)
"""Trainium2 Bass kernel for per-voxel 3x3 SPD matrix logarithm.

Input  x: (2, 9, 64, 128, 128) fp32, channel c = 3*i+j of symmetric M.
Output Y: same shape, Y = U log(S) U^T per voxel.

Branchless spectral method (fp16 end-to-end on device):
  q = tr(M)/3, D = M - q I, p = sqrt(tr(D^2)/6), r = det(D)/(2 p^3)
  phi = acos(clamp(r))/3 via arctan + sin on ACT
  nodes l1<l2<l3 = q + 2p cos(phi + {-, 0, +}2pi/3), gaps floored at dlo
  divided differences c1, c2 of ln at the nodes (self-consistent: gaps are
  the same f16 values used to build the nodes, so cancellation noise in the
  interpolation coefficients is self-limiting)
  Y = c2 D^2 + e1 D + gam I with e1 = c1 + 2 p cos(phi) c2,
  gam = ln(l1) + (q-l1)(c1 + c2 (q-l2))

fp16 device I/O: host converts input to fp16 (6 unique channels) and expands
the 6 fp16 output channels back to the 9-channel fp32 tensor.

Engines: DVE does fp16 TensorTensor (2x mode) / tensor_scalar (4x mode) bulk,
custom DVE ops for clamp+cube and fp16 reciprocal (BITWISE_NOT seed + NR);
ACT does squares, 1/sqrt (Abs_reciprocal_sqrt), arctan/sin, ln; Pool takes
structural 3-channel products (tau, cross, w3, one output mult).
"""
import math
import numpy as np

import concourse.bacc as bacc
import concourse.tile as tile
import concourse.bass as bass
from concourse import mybir
from concourse.bass_utils import run_bass_kernel_spmd

F32 = mybir.dt.float32
F16 = mybir.dt.float16
OP = mybir.AluOpType
AF = mybir.ActivationFunctionType

B = 2
NV = 64 * 128 * 128
NCORE = 8
VPC = NV // NCORE
P = 128
FD = 512
CPB = VPC // (P * FD)        # chunks per batch
NCHUNK = B * CPB
PLANE = VPC // P

CL = 0.99999988
S3 = math.sqrt(3.0)
PI6 = math.pi / 6.0
DLO = 0.01                   # eigen-gap floor (fp16 consistency scale)

# ---- runtime-registered custom DVE ops ----
from concourse import dve_ops as _dvo
from concourse.dve_spec import (
    Spec as _Spec, Src0 as _S0, Src1 as _S1, C0 as _C0, C1 as _C1, C2 as _C2,
    maxx as _maxx, minn as _minn, lower as _lower, _has_src1 as _hs1,
    Bin as _Bin, AluOp as _AluOp,
)
from concourse.dve_uop import DveOpSpec as _DveOpSpec


def _register_dve(name, spec):
    if name in _dvo._SUB_OPCODE_FOR_NAME:
        return next(op for op in _dvo.OPS if op.name == name)
    op = _dvo.DveOp(name, spec, subdim=False, uops_sha={})
    _dvo.OPS.append(op)
    _dvo.CUSTOM_DVE_SPECS[name] = spec
    row = _dvo._CUSTOM_DVE_ROW_BASE + len(_dvo.OPS) - 1
    assert row < 0x20
    _dvo._SUB_OPCODE_FOR_NAME[name] = row
    for ver in ("v3", "v4"):
        uops = _lower(spec, ver=ver)
        res = _DveOpSpec(name=name, opcode=row, uops=uops, rd1_en=_hs1(spec))
        op.uops_sha[ver] = res.sha(ver)
    return op


# r = clamp(in0 * in1^3 * s0, s1, imm2)
DETC_CLAMP = _register_dve("LOGM_DETC_CLAMP", _Spec(
    body=_minn(_maxx(_S0 * (_S1 * _S1 * _S1) * _C0, _C1), _C2),
    reference=lambda in0, in1, s0, s1, imm2: np.minimum(
        np.maximum(in0.astype(np.float32) * (in1.astype(np.float32) ** 3) * s0, s1), imm2
    ).astype(np.float32),
))

# fp16-capable approximate reciprocal (fp32 datapath: BITWISE_NOT seed + 2 NR)
_not_x = _Bin(_AluOp.BITWISE_NOT, _S0, _S0)
_y0 = _not_x * _C0
_y1 = _y0 * (_C1 - _S0 * _y0)


def _ref_recip_fast(in0, in1, c0, c1, c2):
    not_x = (~in0.astype(np.float32).view(np.int32)).view(np.float32)
    y0 = not_x * c0
    y1 = y0 * (c1 - in0.astype(np.float32) * y0)
    return y1 * (c2 - in0.astype(np.float32) * y1)


RECIP16 = _register_dve("LOGM_RECIP16", _Spec(
    body=_y1 * (_C2 - _S0 * _y1), reference=_ref_recip_fast))
RECIP_CONSTS = {"s0": -0.23549792, "s1": 2.0017324, "imm2": 2.0}

# Pin Arctan to trig_and_small (it genuinely lives there) so arctan->sin needs
# one table set; pin Square to abs_reciprocal_sqrt_and_small (square is in
# every set) so squares ride the ARS load. Both only steer set CHOICE.
from concourse import hw_specs as _hw
import concourse.bacc as _bacc_mod
_orig_gat = _hw.get_activation_tables


def _patched_gat(arch):
    t = _orig_gat(arch)
    for sname, fns in t.items():
        if sname != "trig_and_small":
            fns.discard(mybir.ActivationFunctionType.Arctan)
        if sname != "abs_reciprocal_sqrt_and_small":
            fns.discard(mybir.ActivationFunctionType.Square)
    return t


_hw.get_activation_tables = _patched_gat
_bacc_mod.get_activation_tables = _patched_gat

_CACHE = {}


def _register_const(nc, val):
    t = nc.alloc_sbuf_tensor(f"const-f32-{val}", [128, 1], F32)
    nc.gpsimd.memset(t.ap(), val)
    nc.const_aps.aps[(F32, float(val))] = t.ap()


def build():
    nc = bacc.Bacc("TRN2")
    _register_const(nc, PI6)
    _register_const(nc, PI6 + math.pi / 2.0)
    _register_const(nc, 1.0)
    nc.all_engine_barrier()
    xin = nc.dram_tensor("xin", [B, 6, VPC], F16, kind="ExternalInput")
    yout = nc.dram_tensor("yout", [B, 6, VPC], F16, kind="ExternalOutput")

    V, G, S = nc.vector, nc.gpsimd, nc.scalar

    with tile.TileContext(nc) as tc:
        with tc.tile_pool(name="mp", bufs=1) as pool:

            def T2(units, name, dt=F16, bufs=2):
                return pool.tile([P, units * FD], dt, name=name, tag=name, bufs=bufs)

            def b3(ap_fd):
                return ap_fd.unsqueeze(1).broadcast_to((P, 3, FD))

            def b2(ap_fd):
                return ap_fd.unsqueeze(1).broadcast_to((P, 2, FD))

            def c3(t):
                return t[:].rearrange("p (c f) -> p c f", c=3)

            def stageA(ci):
                b, j = divmod(ci, CPB)
                t = {}
                xin_t = T2(6, "xin", bufs=2)
                t["xin"] = xin_t
                src1 = bass.AP(xin, b * 6 * VPC + j * FD,
                               [[PLANE, P], [VPC, 3], [1, FD]])
                src2 = bass.AP(xin, b * 6 * VPC + 3 * VPC + j * FD,
                               [[PLANE, P], [VPC, 3], [1, FD]])
                nc.sync.dma_start(xin_t[:, 0:3 * FD].rearrange("p (c f) -> p c f", c=3), src1)
                nc.sync.dma_start(xin_t[:, 3 * FD:6 * FD].rearrange("p (c f) -> p c f", c=3), src2)
                a_ = xin_t[:, 0 * FD:1 * FD]
                d_ = xin_t[:, 1 * FD:2 * FD]
                f_ = xin_t[:, 2 * FD:3 * FD]
                b_ = xin_t[:, 3 * FD:4 * FD]
                c_ = xin_t[:, 4 * FD:5 * FD]
                e_ = xin_t[:, 5 * FD:6 * FD]
                adf = xin_t[:, 0:3 * FD].rearrange("p (c f) -> p c f", c=3)
                bce = xin_t[:, 3 * FD:6 * FD]

                s1 = T2(1, "s1")
                V.tensor_tensor(s1[:], a_, d_, OP.add)
                V.tensor_tensor(s1[:], s1[:], f_, OP.add)
                q = T2(1, "q", bufs=2)
                V.tensor_scalar(q[:], s1[:], 1.0 / 3.0, None, OP.mult)
                t["q"] = q

                Dt = T2(3, "Dt", bufs=2)
                V.tensor_tensor(c3(Dt), adf, b3(q[:]), OP.subtract)
                t["Dt"] = Dt
                aa = Dt[:, 0:FD]
                dd = Dt[:, FD:2 * FD]

                SQ = T2(6, "SQ", bufs=2)
                S.activation(SQ[:, 0:3 * FD], Dt[:], AF.Square)
                sq_i = S.activation(SQ[:, 3 * FD:6 * FD], bce, AF.Square)
                t["SQ"] = SQ
                t["sq_inst"] = sq_i
                SQb3 = SQ[:, 3 * FD:6 * FD].rearrange("p (c f) -> p c f", c=3)

                stsu = T2(2, "stsu", bufs=2)
                pA = SQ[:].rearrange("p (c f) -> p c f", c=6)
                st2 = stsu[:].rearrange("p (c f) -> p c f", c=2)
                V.tensor_tensor(st2, pA[:, 0:6:3, :], pA[:, 1:6:3, :], OP.add)
                V.tensor_tensor(st2, st2, pA[:, 2:6:3, :], OP.add)
                t["stsu"] = stsu
                p2s6 = T2(1, "p2s6", dt=F32, bufs=2)
                V.scalar_tensor_tensor(p2s6[:], stsu[:, FD:2 * FD], 2.0,
                                       stsu[:, 0:FD], OP.mult, OP.add)
                t["p2s6"] = p2s6

                # det(D): tau_i = Dt_i * SQ_bce_rev_i on Pool
                tau = T2(3, "tau")
                G.tensor_tensor(c3(tau), c3(Dt), SQb3[:, ::-1, :], OP.mult)
                dets = T2(1, "dets")
                V.tensor_tensor(dets[:], tau[:, 0:FD], tau[:, FD:2 * FD], OP.add)
                V.tensor_tensor(dets[:], dets[:], tau[:, 2 * FD:3 * FD], OP.add)
                ad3 = T2(1, "ad3")
                V.tensor_tensor(ad3[:], aa, dd, OP.mult)
                V.tensor_tensor(ad3[:], ad3[:], Dt[:, 2 * FD:3 * FD], OP.mult)
                det = T2(1, "det")
                V.tensor_tensor(det[:], ad3[:], dets[:], OP.subtract)

                cross = T2(3, "cross", bufs=2)
                cb_ap = xin_t[:, 3 * FD:5 * FD].rearrange("p (c f) -> p c f", c=2)[:, ::-1, :]
                G.tensor_tensor(cross[:, 0:2 * FD].rearrange("p (c f) -> p c f", c=2),
                                cb_ap, b2(e_), OP.mult)
                G.tensor_tensor(cross[:, 2 * FD:3 * FD], b_, c_, OP.mult)
                t["cross"] = cross
                bcep = T2(1, "bcep", bufs=2)
                V.tensor_tensor(bcep[:], cross[:, 2 * FD:3 * FD], e_, OP.mult)
                t["bcep"] = bcep
                det32 = T2(1, "det32", dt=F32, bufs=2)
                V.scalar_tensor_tensor(det32[:], bcep[:], 2.0, det[:], OP.mult, OP.add)
                t["det32"] = det32
                return t

            def stageB(ci, t):
                b, j = divmod(ci, CPB)
                xin_t = t["xin"]; Dt = t["Dt"]; SQ = t["SQ"]
                q = t["q"]; p2s6 = t["p2s6"]; cross = t["cross"]
                stsu = t["stsu"]; det32 = t["det32"]
                Dt3 = c3(Dt)
                bce = xin_t[:, 3 * FD:6 * FD]
                su = stsu[:, FD:2 * FD]

                # --- scalar chain ---
                ip = T2(1, "ip", dt=F32)
                ip_i = S.activation(ip[:], p2s6[:], AF.Abs_reciprocal_sqrt, scale=1.0 / 6.0)
                nxt = t.get("next_sq")
                if nxt is not None:
                    ip_i.ins.add_dependency(nxt.ins.name, mybir.DependencyInfo.NO_SYNC_ONLY)
                pt16 = T2(1, "pt16")
                V.scalar_tensor_tensor(pt16[:], p2s6[:], 1.0 / 6.0, ip[:], OP.mult, OP.mult)
                rr = T2(1, "rr")
                V._custom_dve(DETC_CLAMP, out=rr[:], in0=det32[:], in1=ip[:],
                              s0=0.5, s1=-CL, imm2=CL)
                r2 = T2(1, "r2")
                S.activation(r2[:], rr[:], AF.Square)
                isq = T2(1, "isq")
                S.activation(isq[:], r2[:], AF.Abs_reciprocal_sqrt, scale=-1.0, bias=1.0)
                tq = T2(1, "tq")
                V.tensor_tensor(tq[:], rr[:], isq[:], OP.mult)
                at = T2(1, "at", dt=F32)
                S.activation(at[:], tq[:], AF.Arctan)
                sfcf = T2(2, "sfcf")
                S.activation(sfcf[:, 0:FD], at[:], AF.Sin, scale=-1.0 / 3.0, bias=PI6)
                S.activation(sfcf[:, FD:2 * FD], at[:], AF.Sin, scale=-1.0 / 3.0,
                             bias=PI6 + math.pi / 2.0)

                pcps = T2(2, "pcps")
                V.tensor_tensor(pcps[:].rearrange("p (c f) -> p c f", c=2),
                                b2(pt16[:]), sfcf[:].rearrange("p (c f) -> p c f", c=2),
                                OP.mult)
                ps = pcps[:, 0:FD]; pc = pcps[:, FD:2 * FD]
                tsp = T2(1, "tsp")
                V.tensor_scalar(tsp[:], ps, S3, None, OP.mult)
                uu = T2(1, "uu")
                V.tensor_tensor(uu[:], tsp[:], pc, OP.add)

                LD = T2(6, "LD")  # [l1|l2|l3|d12|d23|d13]
                l1 = LD[:, 0:FD]; l2 = LD[:, FD:2 * FD]; l3 = LD[:, 2 * FD:3 * FD]
                d23 = LD[:, 4 * FD:5 * FD]
                V.tensor_tensor(l1, q[:], uu[:], OP.subtract)
                V.tensor_scalar(LD[:, 3 * FD:4 * FD], ps, 2.0 * S3, DLO, OP.mult, OP.max)
                tt3 = T2(1, "tt3")
                V.tensor_scalar(tt3[:], pc, 3.0, None, OP.mult)
                V.tensor_tensor(d23, tt3[:], tsp[:], OP.subtract)
                V.tensor_scalar(d23, d23, 1.0, DLO, OP.mult, OP.max)
                LD6 = LD[:].rearrange("p (c f) -> p c f", c=6)
                # (l2, d13) = (l1, d12) + (d12, d23)
                V.tensor_tensor(LD6[:, 1:6:4, :], LD6[:, 0:4:3, :], LD6[:, 3:5:1, :], OP.add)
                V.tensor_tensor(l3, l2, d23, OP.add)

                ii = T2(3, "ii")
                V._custom_dve(RECIP16, out=ii[:], in0=LD[:, 3 * FD:6 * FD], **RECIP_CONSTS)
                lg = T2(3, "lg")
                S.activation(lg[:], LD[:, 0:3 * FD], AF.Ln)
                gg = T2(2, "gg")
                V.tensor_tensor(gg[:], lg[:, FD:3 * FD], lg[:, 0:2 * FD], OP.subtract)
                c1f = T2(2, "c1f")
                V.tensor_tensor(c1f[:], gg[:], ii[:, 0:2 * FD], OP.mult)
                c1 = c1f[:, 0:FD]
                c2 = T2(1, "c2")
                V.tensor_tensor(c2[:], c1f[:, FD:2 * FD], c1, OP.subtract)
                V.tensor_tensor(c2[:], c2[:], ii[:, 2 * FD:3 * FD], OP.mult)

                # tail: e1 = c1 + 2 pc c2 ; gam = lg1 + uu*(e1 - uu*c2)
                tpc = T2(1, "tpc")
                V.tensor_tensor(tpc[:], pc, c2[:], OP.mult)
                e1 = T2(1, "e1")
                V.scalar_tensor_tensor(e1[:], tpc[:], 2.0, c1, OP.mult, OP.add)
                uc2 = T2(1, "uc2")
                V.tensor_tensor(uc2[:], uu[:], c2[:], OP.mult)
                gam = T2(1, "gam")
                V.tensor_tensor(gam[:], e1[:], uc2[:], OP.subtract)
                V.tensor_tensor(gam[:], uu[:], gam[:], OP.mult)
                V.tensor_tensor(gam[:], gam[:], lg[:, 0:FD], OP.add)

                # --- outputs ---
                # diag: Yd = c2*(SQ_adf + w3) + e1*Dt + gam
                w3 = T2(3, "w3")
                SQb3 = SQ[:, 3 * FD:6 * FD].rearrange("p (c f) -> p c f", c=3)
                G.tensor_tensor(c3(w3), b3(su), SQb3[:, ::-1, :], OP.subtract)
                DD2 = T2(3, "DD2")
                G.tensor_tensor(DD2[:], SQ[:, 0:3 * FD], w3[:], OP.add)
                m1 = T2(3, "m1")
                V.tensor_tensor(c3(m1), c3(DD2), b3(c2[:]), OP.mult)
                m2 = T2(3, "m2")
                G.tensor_tensor(c3(m2), Dt3, b3(e1[:]), OP.mult)
                Yd = T2(3, "Yd", bufs=2)
                V.tensor_tensor(Yd[:], m1[:], m2[:], OP.add)
                V.tensor_tensor(c3(Yd), c3(Yd), b3(gam[:]), OP.add)

                # offdiag: Yo = bce*(e1 - c2*Dt_rev) + c2*cross
                u1 = T2(3, "u1")
                V.tensor_tensor(c3(u1), Dt3[:, ::-1, :], b3(c2[:]), OP.mult)
                V.tensor_tensor(c3(u1), b3(e1[:]), c3(u1), OP.subtract)
                V.tensor_tensor(u1[:], bce, u1[:], OP.mult)
                vv = T2(3, "vv")
                G.tensor_tensor(c3(vv), c3(cross), b3(c2[:]), OP.mult)
                Yo = T2(3, "Yo", bufs=2)
                V.tensor_tensor(Yo[:], u1[:], vv[:], OP.add)

                dstd = bass.AP(yout, b * 6 * VPC + j * FD,
                               [[PLANE, P], [VPC, 3], [1, FD]])
                dsto = bass.AP(yout, b * 6 * VPC + 3 * VPC + j * FD,
                               [[PLANE, P], [VPC, 3], [1, FD]])
                nc.sync.dma_start(dstd, c3(Yd))
                nc.sync.dma_start(dsto, c3(Yo))

            tiles = {0: stageA(0)}
            for ci in range(NCHUNK):
                if ci + 1 < NCHUNK:
                    tiles[ci + 1] = stageA(ci + 1)
                    tiles[ci]["next_sq"] = tiles[ci + 1]["sq_inst"]
                stageB(ci, tiles.pop(ci))
    nc.finalize()
    return nc


def kernel(x):
    x = np.asarray(x)
    xh = x.reshape(B, 9, NV).astype(np.float16)
    sel = [0, 4, 8, 1, 2, 5]  # a d f b c e
    in_maps = []
    for k in range(NCORE):
        shard = np.ascontiguousarray(xh[:, sel, k * VPC:(k + 1) * VPC])
        in_maps.append({"xin": shard})
    if "nc" not in _CACHE:
        _CACHE["nc"] = build()
    res = run_bass_kernel_spmd(_CACHE["nc"], in_maps, core_ids=list(range(NCORE)))
    out = np.empty((B, 9, NV), np.float32)
    for k in range(NCORE):
        sl = slice(k * VPC, (k + 1) * VPC)
        yk = res.results[k]["yout"].astype(np.float32)
        out[:, 0, sl] = yk[:, 0]
        out[:, 4, sl] = yk[:, 1]
        out[:, 8, sl] = yk[:, 2]
        out[:, 1, sl] = yk[:, 3]
        out[:, 3, sl] = yk[:, 3]
        out[:, 2, sl] = yk[:, 4]
        out[:, 6, sl] = yk[:, 4]
        out[:, 5, sl] = yk[:, 5]
        out[:, 7, sl] = yk[:, 5]
    return out.reshape(x.shape)


# revision 15
# speedup vs baseline: 1.2837x; 1.2837x over previous
"""Trainium2 Bass kernel for per-voxel 3x3 SPD matrix logarithm.

Input  x: (2, 9, 64, 128, 128) fp32, channel c = 3*i+j of symmetric M.
Output Y: same shape, Y = U log(S) U^T per voxel.

Branchless spectral method (fp16 end-to-end on device):
  q = tr(M)/3, D = M - q I, p = sqrt(tr(D^2)/6), r = det(D)/(2 p^3)
  phi = acos(clamp(r))/3 via arctan + sin on ACT
  nodes l1<l2<l3 = q + 2p cos(phi + {-, 0, +}2pi/3), gaps floored at dlo
  divided differences c1, c2 of ln at the nodes (self-consistent: gaps are
  the same f16 values used to build the nodes, so cancellation noise in the
  interpolation coefficients is self-limiting)
  Y = c2 D^2 + e1 D + gam I with e1 = c1 + 2 p cos(phi) c2,
  gam = ln(l1) + (q-l1)(c1 + c2 (q-l2))

fp16 device I/O: host converts input to fp16 (6 unique channels) and expands
the 6 fp16 output channels back to the 9-channel fp32 tensor.

Engines: DVE does fp16 TensorTensor (2x mode) / tensor_scalar (4x mode) bulk,
custom DVE ops for clamp+cube and fp16 reciprocal (BITWISE_NOT seed + NR);
ACT does squares, 1/sqrt (Abs_reciprocal_sqrt), arctan/sin, ln; Pool takes
structural 3-channel products (tau, cross, w3, one output mult).
"""
import math
import numpy as np

import concourse.bacc as bacc
import concourse.tile as tile
import concourse.bass as bass
from concourse import mybir
from concourse.bass_utils import run_bass_kernel_spmd

F32 = mybir.dt.float32
F16 = mybir.dt.float16
OP = mybir.AluOpType
AF = mybir.ActivationFunctionType

B = 2
NV = 64 * 128 * 128
NCORE = 8
VPC = NV // NCORE
P = 128
import os
FD = int(os.environ.get("LOGM_FD", "512"))
NBUF = int(os.environ.get("LOGM_NBUF", "2"))
NBUF_B = int(os.environ.get("LOGM_NBUF_B", "2"))
POOL_A = os.environ.get("LOGM_POOL_A", "0") == "1"
TAU_DVE = os.environ.get("LOGM_TAU_DVE", "1") == "1"
NO_POOL = os.environ.get("LOGM_NO_POOL", "0") == "1"
XBUF = int(os.environ.get("LOGM_XBUF", "4"))
CBUF = int(os.environ.get("LOGM_CBUF", "0"))
AHEAD = int(os.environ.get("LOGM_AHEAD", "1"))
PMASK = int(os.environ.get("LOGM_PMASK", "23"))  # 1=cross 2=w3 4=vv 8=m2 16=DD2
CPB = VPC // (P * FD)        # chunks per batch
NCHUNK = B * CPB
PLANE = VPC // P

CL = 0.99999988
S3 = math.sqrt(3.0)
PI6 = math.pi / 6.0
DLO = 0.01                   # eigen-gap floor (fp16 consistency scale)

# ---- runtime-registered custom DVE ops ----
from concourse import dve_ops as _dvo
from concourse.dve_spec import (
    Spec as _Spec, Src0 as _S0, Src1 as _S1, C0 as _C0, C1 as _C1, C2 as _C2,
    maxx as _maxx, minn as _minn, lower as _lower, _has_src1 as _hs1,
    Bin as _Bin, AluOp as _AluOp,
)
from concourse.dve_uop import DveOpSpec as _DveOpSpec


def _register_dve(name, spec):
    if name in _dvo._SUB_OPCODE_FOR_NAME:
        return next(op for op in _dvo.OPS if op.name == name)
    op = _dvo.DveOp(name, spec, subdim=False, uops_sha={})
    _dvo.OPS.append(op)
    _dvo.CUSTOM_DVE_SPECS[name] = spec
    row = _dvo._CUSTOM_DVE_ROW_BASE + len(_dvo.OPS) - 1
    assert row < 0x20
    _dvo._SUB_OPCODE_FOR_NAME[name] = row
    for ver in ("v3", "v4"):
        uops = _lower(spec, ver=ver)
        res = _DveOpSpec(name=name, opcode=row, uops=uops, rd1_en=_hs1(spec))
        op.uops_sha[ver] = res.sha(ver)
    return op


# r = clamp(in0 * in1^3 * s0, s1, imm2)
DETC_CLAMP = _register_dve("LOGM_DETC_CLAMP", _Spec(
    body=_minn(_maxx(_S0 * (_S1 * _S1 * _S1) * _C0, _C1), _C2),
    reference=lambda in0, in1, s0, s1, imm2: np.minimum(
        np.maximum(in0.astype(np.float32) * (in1.astype(np.float32) ** 3) * s0, s1), imm2
    ).astype(np.float32),
))

# fp16-capable approximate reciprocal (fp32 datapath: BITWISE_NOT seed + 2 NR)
_not_x = _Bin(_AluOp.BITWISE_NOT, _S0, _S0)
_y0 = _not_x * _C0
_y1 = _y0 * (_C1 - _S0 * _y0)


def _ref_recip_fast(in0, in1, c0, c1, c2):
    not_x = (~in0.astype(np.float32).view(np.int32)).view(np.float32)
    y0 = not_x * c0
    y1 = y0 * (c1 - in0.astype(np.float32) * y0)
    return y1 * (c2 - in0.astype(np.float32) * y1)


RECIP16 = _register_dve("LOGM_RECIP16", _Spec(
    body=_y1 * (_C2 - _S0 * _y1), reference=_ref_recip_fast))
RECIP_CONSTS = {"s0": -0.23549792, "s1": 2.0017324, "imm2": 2.0}

# Pin Arctan to trig_and_small (it genuinely lives there) so arctan->sin needs
# one table set; pin Square to abs_reciprocal_sqrt_and_small (square is in
# every set) so squares ride the ARS load. Both only steer set CHOICE.
from concourse import hw_specs as _hw
import concourse.bacc as _bacc_mod
_orig_gat = _hw.get_activation_tables


def _patched_gat(arch):
    t = _orig_gat(arch)
    for sname, fns in t.items():
        if sname != "trig_and_small":
            fns.discard(mybir.ActivationFunctionType.Arctan)
        if sname != "abs_reciprocal_sqrt_and_small":
            fns.discard(mybir.ActivationFunctionType.Square)
    return t


_hw.get_activation_tables = _patched_gat
_bacc_mod.get_activation_tables = _patched_gat

_CACHE = {}


def _register_const(nc, val):
    t = nc.alloc_sbuf_tensor(f"const-f32-{val}", [128, 1], F32)
    nc.gpsimd.memset(t.ap(), val)
    nc.const_aps.aps[(F32, float(val))] = t.ap()


def build():
    nc = bacc.Bacc("TRN2")
    _register_const(nc, PI6)
    _register_const(nc, PI6 + math.pi / 2.0)
    _register_const(nc, 1.0)
    nc.all_engine_barrier()
    xin = nc.dram_tensor("xin", [B, 6, VPC], F16, kind="ExternalInput")
    yout = nc.dram_tensor("yout", [B, 6, VPC], F16, kind="ExternalOutput")

    V, S = nc.vector, nc.scalar
    G = nc.vector if NO_POOL else nc.gpsimd

    with tile.TileContext(nc) as tc:
        with tc.tile_pool(name="mp", bufs=1) as pool:

            def T2(units, name, dt=F16, bufs=None):
                bufs = NBUF if bufs is None else bufs
                return pool.tile([P, units * FD], dt, name=name, tag=name, bufs=bufs)

            def b3(ap_fd):
                return ap_fd.unsqueeze(1).broadcast_to((P, 3, FD))

            def b2(ap_fd):
                return ap_fd.unsqueeze(1).broadcast_to((P, 2, FD))

            def c3(t):
                return t[:].rearrange("p (c f) -> p c f", c=3)

            def stageA(ci):
                b, j = divmod(ci, CPB)
                t = {}
                xin_t = T2(6, "xin", bufs=(XBUF or NBUF))
                t["xin"] = xin_t
                src1 = bass.AP(xin, b * 6 * VPC + j * FD,
                               [[PLANE, P], [VPC, 3], [1, FD]])
                src2 = bass.AP(xin, b * 6 * VPC + 3 * VPC + j * FD,
                               [[PLANE, P], [VPC, 3], [1, FD]])
                nc.sync.dma_start(xin_t[:, 0:3 * FD].rearrange("p (c f) -> p c f", c=3), src1)
                nc.sync.dma_start(xin_t[:, 3 * FD:6 * FD].rearrange("p (c f) -> p c f", c=3), src2)
                a_ = xin_t[:, 0 * FD:1 * FD]
                d_ = xin_t[:, 1 * FD:2 * FD]
                f_ = xin_t[:, 2 * FD:3 * FD]
                b_ = xin_t[:, 3 * FD:4 * FD]
                c_ = xin_t[:, 4 * FD:5 * FD]
                e_ = xin_t[:, 5 * FD:6 * FD]
                adf = xin_t[:, 0:3 * FD].rearrange("p (c f) -> p c f", c=3)
                bce = xin_t[:, 3 * FD:6 * FD]

                s1 = T2(1, "s1")
                E1 = G if POOL_A else V
                E1.tensor_tensor(s1[:], a_, d_, OP.add)
                E1.tensor_tensor(s1[:], s1[:], f_, OP.add)
                q = T2(1, "q", bufs=(CBUF or NBUF))
                V.tensor_scalar(q[:], s1[:], 1.0 / 3.0, None, OP.mult)
                t["q"] = q

                Dt = T2(3, "Dt", bufs=(CBUF or NBUF))
                V.tensor_tensor(c3(Dt), adf, b3(q[:]), OP.subtract)
                t["Dt"] = Dt
                aa = Dt[:, 0:FD]
                dd = Dt[:, FD:2 * FD]

                SQ = T2(6, "SQ", bufs=(CBUF or NBUF))
                S.activation(SQ[:, 0:3 * FD], Dt[:], AF.Square)
                sq_i = S.activation(SQ[:, 3 * FD:6 * FD], bce, AF.Square)
                t["SQ"] = SQ
                t["sq_inst"] = sq_i
                SQb3 = SQ[:, 3 * FD:6 * FD].rearrange("p (c f) -> p c f", c=3)

                stsu = T2(2, "stsu", bufs=(CBUF or NBUF))
                pA = SQ[:].rearrange("p (c f) -> p c f", c=6)
                st2 = stsu[:].rearrange("p (c f) -> p c f", c=2)
                V.tensor_tensor(st2, pA[:, 0:6:3, :], pA[:, 1:6:3, :], OP.add)
                V.tensor_tensor(st2, st2, pA[:, 2:6:3, :], OP.add)
                t["stsu"] = stsu
                p2s6 = T2(1, "p2s6", dt=F32, bufs=None)
                V.scalar_tensor_tensor(p2s6[:], stsu[:, FD:2 * FD], 2.0,
                                       stsu[:, 0:FD], OP.mult, OP.add)
                t["p2s6"] = p2s6

                # det(D): tau_i = Dt_i * SQ_bce_rev_i on Pool
                tau = T2(3, "tau")
                (V if TAU_DVE else G).tensor_tensor(c3(tau), c3(Dt), SQb3[:, ::-1, :], OP.mult)
                dets = T2(1, "dets")
                EA = G if POOL_A else V
                EA.tensor_tensor(dets[:], tau[:, 0:FD], tau[:, FD:2 * FD], OP.add)
                EA.tensor_tensor(dets[:], dets[:], tau[:, 2 * FD:3 * FD], OP.add)
                ad3 = T2(1, "ad3")
                EA.tensor_tensor(ad3[:], aa, dd, OP.mult)
                EA.tensor_tensor(ad3[:], ad3[:], Dt[:, 2 * FD:3 * FD], OP.mult)
                det = T2(1, "det")
                V.tensor_tensor(det[:], ad3[:], dets[:], OP.subtract)

                cross = T2(3, "cross", bufs=(CBUF or NBUF))
                cb_ap = xin_t[:, 3 * FD:5 * FD].rearrange("p (c f) -> p c f", c=2)[:, ::-1, :]
                EC = G if (PMASK & 1) else V
                EC.tensor_tensor(cross[:, 0:2 * FD].rearrange("p (c f) -> p c f", c=2),
                                cb_ap, b2(e_), OP.mult)
                EC.tensor_tensor(cross[:, 2 * FD:3 * FD], b_, c_, OP.mult)
                t["cross"] = cross
                bcep = T2(1, "bcep", bufs=None)
                V.tensor_tensor(bcep[:], cross[:, 2 * FD:3 * FD], e_, OP.mult)
                t["bcep"] = bcep
                det32 = T2(1, "det32", dt=F32, bufs=None)
                V.scalar_tensor_tensor(det32[:], bcep[:], 2.0, det[:], OP.mult, OP.add)
                t["det32"] = det32
                return t

            def stageB(ci, t):
                def T2B(units, name, dt=F16):
                    return pool.tile([P, units * FD], dt, name=name, tag=name,
                                     bufs=NBUF_B)
                b, j = divmod(ci, CPB)
                xin_t = t["xin"]; Dt = t["Dt"]; SQ = t["SQ"]
                q = t["q"]; p2s6 = t["p2s6"]; cross = t["cross"]
                stsu = t["stsu"]; det32 = t["det32"]
                Dt3 = c3(Dt)
                bce = xin_t[:, 3 * FD:6 * FD]
                su = stsu[:, FD:2 * FD]

                # --- scalar chain ---
                ip = T2B(1, "ip", dt=F32)
                ip_i = S.activation(ip[:], p2s6[:], AF.Abs_reciprocal_sqrt, scale=1.0 / 6.0)
                nxt = t.get("next_sq")
                if nxt is not None:
                    pass
                pt16 = T2B(1, "pt16")
                V.scalar_tensor_tensor(pt16[:], p2s6[:], 1.0 / 6.0, ip[:], OP.mult, OP.mult)
                rr = T2B(1, "rr")
                V._custom_dve(DETC_CLAMP, out=rr[:], in0=det32[:], in1=ip[:],
                              s0=0.5, s1=-CL, imm2=CL)
                r2 = T2B(1, "r2")
                S.activation(r2[:], rr[:], AF.Square)
                isq = T2B(1, "isq")
                S.activation(isq[:], r2[:], AF.Abs_reciprocal_sqrt, scale=-1.0, bias=1.0)
                tq = T2B(1, "tq")
                V.tensor_tensor(tq[:], rr[:], isq[:], OP.mult)
                at = T2B(1, "at", dt=F32)
                S.activation(at[:], tq[:], AF.Arctan)
                sfcf = T2B(2, "sfcf")
                S.activation(sfcf[:, 0:FD], at[:], AF.Sin, scale=-1.0 / 3.0, bias=PI6)
                S.activation(sfcf[:, FD:2 * FD], at[:], AF.Sin, scale=-1.0 / 3.0,
                             bias=PI6 + math.pi / 2.0)

                pcps = T2B(2, "pcps")
                V.tensor_tensor(pcps[:].rearrange("p (c f) -> p c f", c=2),
                                b2(pt16[:]), sfcf[:].rearrange("p (c f) -> p c f", c=2),
                                OP.mult)
                ps = pcps[:, 0:FD]; pc = pcps[:, FD:2 * FD]
                tsp = T2B(1, "tsp")
                V.tensor_scalar(tsp[:], ps, S3, None, OP.mult)
                uu = T2B(1, "uu")
                V.tensor_tensor(uu[:], tsp[:], pc, OP.add)

                LD = T2B(6, "LD")  # [l1|l2|l3|d12|d23|d13]
                l1 = LD[:, 0:FD]; l2 = LD[:, FD:2 * FD]; l3 = LD[:, 2 * FD:3 * FD]
                d23 = LD[:, 4 * FD:5 * FD]
                V.tensor_tensor(l1, q[:], uu[:], OP.subtract)
                V.tensor_scalar(LD[:, 3 * FD:4 * FD], ps, 2.0 * S3, DLO, OP.mult, OP.max)
                tt3 = T2B(1, "tt3")
                V.tensor_scalar(tt3[:], pc, 3.0, None, OP.mult)
                V.tensor_tensor(d23, tt3[:], tsp[:], OP.subtract)
                V.tensor_scalar(d23, d23, 1.0, DLO, OP.mult, OP.max)
                LD6 = LD[:].rearrange("p (c f) -> p c f", c=6)
                # (l2, d13) = (l1, d12) + (d12, d23)
                V.tensor_tensor(LD6[:, 1:6:4, :], LD6[:, 0:4:3, :], LD6[:, 3:5:1, :], OP.add)
                V.tensor_tensor(l3, l2, d23, OP.add)

                ii = T2B(3, "ii")
                V._custom_dve(RECIP16, out=ii[:], in0=LD[:, 3 * FD:6 * FD], **RECIP_CONSTS)
                lg = T2B(3, "lg")
                S.activation(lg[:], LD[:, 0:3 * FD], AF.Ln)
                gg = T2B(2, "gg")
                V.tensor_tensor(gg[:], lg[:, FD:3 * FD], lg[:, 0:2 * FD], OP.subtract)
                c1f = T2B(2, "c1f")
                V.tensor_tensor(c1f[:], gg[:], ii[:, 0:2 * FD], OP.mult)
                c1 = c1f[:, 0:FD]
                c2 = T2B(1, "c2")
                V.tensor_tensor(c2[:], c1f[:, FD:2 * FD], c1, OP.subtract)
                V.tensor_tensor(c2[:], c2[:], ii[:, 2 * FD:3 * FD], OP.mult)

                # tail: e1 = c1 + 2 pc c2 ; gam = lg1 + uu*(e1 - uu*c2)
                tpc = T2B(1, "tpc")
                V.tensor_tensor(tpc[:], pc, c2[:], OP.mult)
                e1 = T2B(1, "e1")
                V.scalar_tensor_tensor(e1[:], tpc[:], 2.0, c1, OP.mult, OP.add)
                uc2 = T2B(1, "uc2")
                V.tensor_tensor(uc2[:], uu[:], c2[:], OP.mult)
                gam = T2B(1, "gam")
                V.tensor_tensor(gam[:], e1[:], uc2[:], OP.subtract)
                V.tensor_tensor(gam[:], uu[:], gam[:], OP.mult)
                V.tensor_tensor(gam[:], gam[:], lg[:, 0:FD], OP.add)

                # --- outputs ---
                # diag: Yd = c2*(SQ_adf + w3) + e1*Dt + gam
                w3 = T2B(3, "w3")
                SQb3 = SQ[:, 3 * FD:6 * FD].rearrange("p (c f) -> p c f", c=3)
                (G if (PMASK & 2) else V).tensor_tensor(c3(w3), b3(su), SQb3[:, ::-1, :], OP.subtract)
                DD2 = T2B(3, "DD2")
                (G if (PMASK & 16) else V).tensor_tensor(DD2[:], SQ[:, 0:3 * FD], w3[:], OP.add)
                V.tensor_tensor(c3(DD2), c3(DD2), b3(c2[:]), OP.mult)
                m1 = DD2
                m2 = T2B(3, "m2")
                (G if (PMASK & 8) else V).tensor_tensor(c3(m2), Dt3, b3(e1[:]), OP.mult)
                Yd = T2B(3, "Yd")
                V.tensor_tensor(Yd[:], m1[:], m2[:], OP.add)
                V.tensor_tensor(c3(Yd), c3(Yd), b3(gam[:]), OP.add)

                # offdiag: Yo = bce*(e1 - c2*Dt_rev) + c2*cross
                u1 = T2B(3, "u1")
                V.tensor_tensor(c3(u1), Dt3[:, ::-1, :], b3(c2[:]), OP.mult)
                V.tensor_tensor(c3(u1), b3(e1[:]), c3(u1), OP.subtract)
                V.tensor_tensor(u1[:], bce, u1[:], OP.mult)
                vv = T2B(3, "vv")
                (G if (PMASK & 4) else V).tensor_tensor(c3(vv), c3(cross), b3(c2[:]), OP.mult)
                Yo = T2B(3, "Yo")
                V.tensor_tensor(Yo[:], u1[:], vv[:], OP.add)

                dstd = bass.AP(yout, b * 6 * VPC + j * FD,
                               [[PLANE, P], [VPC, 3], [1, FD]])
                dsto = bass.AP(yout, b * 6 * VPC + 3 * VPC + j * FD,
                               [[PLANE, P], [VPC, 3], [1, FD]])
                nc.sync.dma_start(dstd, c3(Yd))
                nc.sync.dma_start(dsto, c3(Yo))

            tiles = {}
            for k in range(min(AHEAD, NCHUNK)):
                tiles[k] = stageA(k)
            for ci in range(NCHUNK):
                if ci + AHEAD < NCHUNK:
                    tiles[ci + AHEAD] = stageA(ci + AHEAD)
                stageB(ci, tiles.pop(ci))
    nc.finalize()
    return nc


def kernel(x):
    x = np.asarray(x)
    xh = x.reshape(B, 9, NV).astype(np.float16)
    sel = [0, 4, 8, 1, 2, 5]  # a d f b c e
    in_maps = []
    for k in range(NCORE):
        shard = np.ascontiguousarray(xh[:, sel, k * VPC:(k + 1) * VPC])
        in_maps.append({"xin": shard})
    if "nc" not in _CACHE:
        _CACHE["nc"] = build()
    res = run_bass_kernel_spmd(_CACHE["nc"], in_maps, core_ids=list(range(NCORE)))
    out = np.empty((B, 9, NV), np.float32)
    for k in range(NCORE):
        sl = slice(k * VPC, (k + 1) * VPC)
        yk = res.results[k]["yout"].astype(np.float32)
        out[:, 0, sl] = yk[:, 0]
        out[:, 4, sl] = yk[:, 1]
        out[:, 8, sl] = yk[:, 2]
        out[:, 1, sl] = yk[:, 3]
        out[:, 3, sl] = yk[:, 3]
        out[:, 2, sl] = yk[:, 4]
        out[:, 6, sl] = yk[:, 4]
        out[:, 5, sl] = yk[:, 5]
        out[:, 7, sl] = yk[:, 5]
    return out.reshape(x.shape)


# revision 19
# speedup vs baseline: 1.3087x; 1.0194x over previous
"""Trainium2 Bass kernel for per-voxel 3x3 SPD matrix logarithm.

Input  x: (2, 9, 64, 128, 128) fp32, channel c = 3*i+j of symmetric M.
Output Y: same shape, Y = U log(S) U^T per voxel.

Branchless spectral method (fp16 end-to-end on device):
  q = tr(M)/3, D = M - q I, p = sqrt(tr(D^2)/6), r = det(D)/(2 p^3)
  phi = acos(clamp(r))/3 via arctan + sin on ACT
  nodes l1<l2<l3 = q + 2p cos(phi + {-, 0, +}2pi/3), gaps floored at dlo
  divided differences c1, c2 of ln at the nodes (self-consistent: gaps are
  the same f16 values used to build the nodes, so cancellation noise in the
  interpolation coefficients is self-limiting)
  Y = c2 D^2 + e1 D + gam I with e1 = c1 + 2 p cos(phi) c2,
  gam = ln(l1) + (q-l1)(c1 + c2 (q-l2))

fp16 device I/O: host converts input to fp16 (6 unique channels) and expands
the 6 fp16 output channels back to the 9-channel fp32 tensor.

Engines: DVE does fp16 TensorTensor (2x mode) / tensor_scalar (4x mode) bulk,
custom DVE ops for clamp+cube and fp16 reciprocal (BITWISE_NOT seed + NR);
ACT does squares, 1/sqrt (Abs_reciprocal_sqrt), arctan/sin, ln; Pool takes
structural 3-channel products (tau, cross, w3, one output mult).
"""
import math
import numpy as np

import concourse.bacc as bacc
import concourse.tile as tile
import concourse.bass as bass
from concourse import mybir
from concourse.bass_utils import run_bass_kernel_spmd

F32 = mybir.dt.float32
F16 = mybir.dt.float16
OP = mybir.AluOpType
AF = mybir.ActivationFunctionType

B = 2
NV = 64 * 128 * 128
NCORE = 8
VPC = NV // NCORE
P = 128
import os
FD = int(os.environ.get("LOGM_FD", "512"))
NBUF = int(os.environ.get("LOGM_NBUF", "2"))
NBUF_B = int(os.environ.get("LOGM_NBUF_B", "2"))
POOL_A = os.environ.get("LOGM_POOL_A", "0") == "1"
TAU_DVE = os.environ.get("LOGM_TAU_DVE", "1") == "1"
NO_POOL = os.environ.get("LOGM_NO_POOL", "0") == "1"
XBUF = int(os.environ.get("LOGM_XBUF", "4"))
CBUF = int(os.environ.get("LOGM_CBUF", "3"))
AHEAD = int(os.environ.get("LOGM_AHEAD", "1"))
PMASK = int(os.environ.get("LOGM_PMASK", "23"))  # 1=cross 2=w3 4=vv 8=m2 16=DD2
CPB = VPC // (P * FD)        # chunks per batch
NCHUNK = B * CPB
PLANE = VPC // P

CL = 0.99999988
S3 = math.sqrt(3.0)
PI6 = math.pi / 6.0
DLO = 0.01                   # eigen-gap floor (fp16 consistency scale)

# ---- runtime-registered custom DVE ops ----
from concourse import dve_ops as _dvo
from concourse.dve_spec import (
    Spec as _Spec, Src0 as _S0, Src1 as _S1, C0 as _C0, C1 as _C1, C2 as _C2,
    maxx as _maxx, minn as _minn, lower as _lower, _has_src1 as _hs1,
    Bin as _Bin, AluOp as _AluOp,
)
from concourse.dve_uop import DveOpSpec as _DveOpSpec


def _register_dve(name, spec):
    if name in _dvo._SUB_OPCODE_FOR_NAME:
        return next(op for op in _dvo.OPS if op.name == name)
    op = _dvo.DveOp(name, spec, subdim=False, uops_sha={})
    _dvo.OPS.append(op)
    _dvo.CUSTOM_DVE_SPECS[name] = spec
    row = _dvo._CUSTOM_DVE_ROW_BASE + len(_dvo.OPS) - 1
    assert row < 0x20
    _dvo._SUB_OPCODE_FOR_NAME[name] = row
    for ver in ("v3", "v4"):
        uops = _lower(spec, ver=ver)
        res = _DveOpSpec(name=name, opcode=row, uops=uops, rd1_en=_hs1(spec))
        op.uops_sha[ver] = res.sha(ver)
    return op


# r = clamp(in0 * in1^3 * s0, s1, imm2)
DETC_CLAMP = _register_dve("LOGM_DETC_CLAMP", _Spec(
    body=_minn(_maxx(_S0 * (_S1 * _S1 * _S1) * _C0, _C1), _C2),
    reference=lambda in0, in1, s0, s1, imm2: np.minimum(
        np.maximum(in0.astype(np.float32) * (in1.astype(np.float32) ** 3) * s0, s1), imm2
    ).astype(np.float32),
))

# fp16-capable approximate reciprocal (fp32 datapath: BITWISE_NOT seed + 2 NR)
_not_x = _Bin(_AluOp.BITWISE_NOT, _S0, _S0)
_y0 = _not_x * _C0
_y1 = _y0 * (_C1 - _S0 * _y0)


def _ref_recip_fast(in0, in1, c0, c1, c2):
    not_x = (~in0.astype(np.float32).view(np.int32)).view(np.float32)
    y0 = not_x * c0
    y1 = y0 * (c1 - in0.astype(np.float32) * y0)
    return y1 * (c2 - in0.astype(np.float32) * y1)


RECIP16 = _register_dve("LOGM_RECIP16", _Spec(
    body=_y1 * (_C2 - _S0 * _y1), reference=_ref_recip_fast))
RECIP_CONSTS = {"s0": -0.23549792, "s1": 2.0017324, "imm2": 2.0}

# Pin Arctan to trig_and_small (it genuinely lives there) so arctan->sin needs
# one table set; pin Square to abs_reciprocal_sqrt_and_small (square is in
# every set) so squares ride the ARS load. Both only steer set CHOICE.
from concourse import hw_specs as _hw
import concourse.bacc as _bacc_mod
_orig_gat = _hw.get_activation_tables


def _patched_gat(arch):
    t = _orig_gat(arch)
    for sname, fns in t.items():
        if sname != "trig_and_small":
            fns.discard(mybir.ActivationFunctionType.Arctan)
        if sname != "abs_reciprocal_sqrt_and_small":
            fns.discard(mybir.ActivationFunctionType.Square)
    return t


_hw.get_activation_tables = _patched_gat
_bacc_mod.get_activation_tables = _patched_gat

_CACHE = {}


def _register_const(nc, val):
    t = nc.alloc_sbuf_tensor(f"const-f32-{val}", [128, 1], F32)
    nc.gpsimd.memset(t.ap(), val)
    nc.const_aps.aps[(F32, float(val))] = t.ap()


def build():
    nc = bacc.Bacc("TRN2")
    _register_const(nc, PI6)
    _register_const(nc, PI6 + math.pi / 2.0)
    _register_const(nc, 1.0)
    nc.all_engine_barrier()
    xin = nc.dram_tensor("xin", [B, 6, VPC], F16, kind="ExternalInput")
    yout = nc.dram_tensor("yout", [B, 6, VPC], F16, kind="ExternalOutput")

    V, S = nc.vector, nc.scalar
    G = nc.vector if NO_POOL else nc.gpsimd

    with tile.TileContext(nc) as tc:
        with tc.tile_pool(name="mp", bufs=1) as pool:

            def T2(units, name, dt=F16, bufs=None):
                bufs = NBUF if bufs is None else bufs
                return pool.tile([P, units * FD], dt, name=name, tag=name, bufs=bufs)

            def b3(ap_fd):
                return ap_fd.unsqueeze(1).broadcast_to((P, 3, FD))

            def b2(ap_fd):
                return ap_fd.unsqueeze(1).broadcast_to((P, 2, FD))

            def c3(t):
                return t[:].rearrange("p (c f) -> p c f", c=3)

            def stageA(ci):
                b, j = divmod(ci, CPB)
                t = {}
                xin_t = T2(6, "xin", bufs=(XBUF or NBUF))
                t["xin"] = xin_t
                src1 = bass.AP(xin, b * 6 * VPC + j * FD,
                               [[PLANE, P], [VPC, 3], [1, FD]])
                src2 = bass.AP(xin, b * 6 * VPC + 3 * VPC + j * FD,
                               [[PLANE, P], [VPC, 3], [1, FD]])
                nc.sync.dma_start(xin_t[:, 0:3 * FD].rearrange("p (c f) -> p c f", c=3), src1)
                nc.sync.dma_start(xin_t[:, 3 * FD:6 * FD].rearrange("p (c f) -> p c f", c=3), src2)
                a_ = xin_t[:, 0 * FD:1 * FD]
                d_ = xin_t[:, 1 * FD:2 * FD]
                f_ = xin_t[:, 2 * FD:3 * FD]
                b_ = xin_t[:, 3 * FD:4 * FD]
                c_ = xin_t[:, 4 * FD:5 * FD]
                e_ = xin_t[:, 5 * FD:6 * FD]
                adf = xin_t[:, 0:3 * FD].rearrange("p (c f) -> p c f", c=3)
                bce = xin_t[:, 3 * FD:6 * FD]

                s1 = T2(1, "s1")
                E1 = G if POOL_A else V
                E1.tensor_tensor(s1[:], a_, d_, OP.add)
                E1.tensor_tensor(s1[:], s1[:], f_, OP.add)
                q = T2(1, "q", bufs=(CBUF or NBUF))
                V.tensor_scalar(q[:], s1[:], 1.0 / 3.0, None, OP.mult)
                t["q"] = q

                Dt = T2(3, "Dt", bufs=(CBUF or NBUF))
                V.tensor_tensor(c3(Dt), adf, b3(q[:]), OP.subtract)
                t["Dt"] = Dt
                aa = Dt[:, 0:FD]
                dd = Dt[:, FD:2 * FD]

                SQ = T2(6, "SQ", bufs=(CBUF or NBUF))
                S.activation(SQ[:, 0:3 * FD], Dt[:], AF.Square)
                sq_i = S.activation(SQ[:, 3 * FD:6 * FD], bce, AF.Square)
                t["SQ"] = SQ
                t["sq_inst"] = sq_i
                SQb3 = SQ[:, 3 * FD:6 * FD].rearrange("p (c f) -> p c f", c=3)

                stsu = T2(2, "stsu", bufs=(CBUF or NBUF))
                pA = SQ[:].rearrange("p (c f) -> p c f", c=6)
                st2 = stsu[:].rearrange("p (c f) -> p c f", c=2)
                V.tensor_tensor(st2, pA[:, 0:6:3, :], pA[:, 1:6:3, :], OP.add)
                V.tensor_tensor(st2, st2, pA[:, 2:6:3, :], OP.add)
                t["stsu"] = stsu
                p2s6 = T2(1, "p2s6", bufs=(CBUF or NBUF))
                V.scalar_tensor_tensor(p2s6[:], stsu[:, FD:2 * FD], 2.0,
                                       stsu[:, 0:FD], OP.mult, OP.add)
                t["p2s6"] = p2s6

                # det(D): tau_i = Dt_i * SQ_bce_rev_i on Pool
                tau = T2(3, "tau")
                (V if TAU_DVE else G).tensor_tensor(c3(tau), c3(Dt), SQb3[:, ::-1, :], OP.mult)
                dets = T2(1, "dets")
                EA = G if POOL_A else V
                EA.tensor_tensor(dets[:], tau[:, 0:FD], tau[:, FD:2 * FD], OP.add)
                EA.tensor_tensor(dets[:], dets[:], tau[:, 2 * FD:3 * FD], OP.add)
                ad3 = T2(1, "ad3")
                EA.tensor_tensor(ad3[:], aa, dd, OP.mult)
                EA.tensor_tensor(ad3[:], ad3[:], Dt[:, 2 * FD:3 * FD], OP.mult)
                det = T2(1, "det")
                V.tensor_tensor(det[:], ad3[:], dets[:], OP.subtract)

                cross = T2(3, "cross", bufs=(CBUF or NBUF))
                cb_ap = xin_t[:, 3 * FD:5 * FD].rearrange("p (c f) -> p c f", c=2)[:, ::-1, :]
                EC = G if (PMASK & 1) else V
                EC.tensor_tensor(cross[:, 0:2 * FD].rearrange("p (c f) -> p c f", c=2),
                                cb_ap, b2(e_), OP.mult)
                EC.tensor_tensor(cross[:, 2 * FD:3 * FD], b_, c_, OP.mult)
                t["cross"] = cross
                bcep = T2(1, "bcep", bufs=(CBUF or NBUF))
                V.tensor_tensor(bcep[:], cross[:, 2 * FD:3 * FD], e_, OP.mult)
                t["bcep"] = bcep
                det32 = T2(1, "det32", bufs=(CBUF or NBUF))
                V.tensor_scalar(det32[:], bcep[:], 2.0, None, OP.mult)
                V.tensor_tensor(det32[:], det32[:], det[:], OP.add)
                t["det32"] = det32
                return t

            def stageB(ci, t):
                def T2B(units, name, dt=F16):
                    return pool.tile([P, units * FD], dt, name=name, tag=name,
                                     bufs=NBUF_B)
                b, j = divmod(ci, CPB)
                xin_t = t["xin"]; Dt = t["Dt"]; SQ = t["SQ"]
                q = t["q"]; p2s6 = t["p2s6"]; cross = t["cross"]
                stsu = t["stsu"]; det32 = t["det32"]
                Dt3 = c3(Dt)
                bce = xin_t[:, 3 * FD:6 * FD]
                su = stsu[:, FD:2 * FD]

                # --- scalar chain ---
                ip = T2B(1, "ip")
                ip_i = S.activation(ip[:], p2s6[:], AF.Abs_reciprocal_sqrt, scale=1.0 / 6.0)
                nxt = t.get("next_sq")
                if nxt is not None:
                    pass
                pt16 = T2B(1, "pt16")
                V.tensor_tensor(pt16[:], p2s6[:], ip[:], OP.mult)
                V.tensor_scalar(pt16[:], pt16[:], 1.0 / 6.0, None, OP.mult)
                rr = T2B(1, "rr")
                V._custom_dve(DETC_CLAMP, out=rr[:], in0=det32[:], in1=ip[:],
                              s0=0.5, s1=-CL, imm2=CL)
                r2 = T2B(1, "r2")
                S.activation(r2[:], rr[:], AF.Square)
                isq = T2B(1, "isq")
                S.activation(isq[:], r2[:], AF.Abs_reciprocal_sqrt, scale=-1.0, bias=1.0)
                tq = T2B(1, "tq")
                V.tensor_tensor(tq[:], rr[:], isq[:], OP.mult)
                at = T2B(1, "at", dt=F32)
                S.activation(at[:], tq[:], AF.Arctan)
                sfcf = T2B(2, "sfcf")
                S.activation(sfcf[:, 0:FD], at[:], AF.Sin, scale=-1.0 / 3.0, bias=PI6)
                S.activation(sfcf[:, FD:2 * FD], at[:], AF.Sin, scale=-1.0 / 3.0,
                             bias=PI6 + math.pi / 2.0)

                pcps = T2B(2, "pcps")
                V.tensor_tensor(pcps[:].rearrange("p (c f) -> p c f", c=2),
                                b2(pt16[:]), sfcf[:].rearrange("p (c f) -> p c f", c=2),
                                OP.mult)
                ps = pcps[:, 0:FD]; pc = pcps[:, FD:2 * FD]
                tsp = T2B(1, "tsp")
                V.tensor_scalar(tsp[:], ps, S3, None, OP.mult)
                uu = T2B(1, "uu")
                V.tensor_tensor(uu[:], tsp[:], pc, OP.add)

                LD = T2B(6, "LD")  # [l1|l2|l3|d12|d23|d13]
                l1 = LD[:, 0:FD]; l2 = LD[:, FD:2 * FD]; l3 = LD[:, 2 * FD:3 * FD]
                d23 = LD[:, 4 * FD:5 * FD]
                V.tensor_tensor(l1, q[:], uu[:], OP.subtract)
                V.tensor_scalar(LD[:, 3 * FD:4 * FD], ps, 2.0 * S3, DLO, OP.mult, OP.max)
                tt3 = T2B(1, "tt3")
                V.tensor_scalar(tt3[:], pc, 3.0, None, OP.mult)
                V.tensor_tensor(d23, tt3[:], tsp[:], OP.subtract)
                V.tensor_scalar(d23, d23, 1.0, DLO, OP.mult, OP.max)
                LD6 = LD[:].rearrange("p (c f) -> p c f", c=6)
                # (l2, d13) = (l1, d12) + (d12, d23)
                V.tensor_tensor(LD6[:, 1:6:4, :], LD6[:, 0:4:3, :], LD6[:, 3:5:1, :], OP.add)
                V.tensor_tensor(l3, l2, d23, OP.add)

                ii = T2B(3, "ii")
                V._custom_dve(RECIP16, out=ii[:], in0=LD[:, 3 * FD:6 * FD], **RECIP_CONSTS)
                lg = T2B(3, "lg")
                S.activation(lg[:], LD[:, 0:3 * FD], AF.Ln)
                gg = T2B(2, "gg")
                V.tensor_tensor(gg[:], lg[:, FD:3 * FD], lg[:, 0:2 * FD], OP.subtract)
                c1f = T2B(2, "c1f")
                V.tensor_tensor(c1f[:], gg[:], ii[:, 0:2 * FD], OP.mult)
                c1 = c1f[:, 0:FD]
                c2 = T2B(1, "c2")
                V.tensor_tensor(c2[:], c1f[:, FD:2 * FD], c1, OP.subtract)
                V.tensor_tensor(c2[:], c2[:], ii[:, 2 * FD:3 * FD], OP.mult)

                # tail: e1 = c1 + 2 pc c2 ; gam = lg1 + uu*(e1 - uu*c2)
                tpc = T2B(1, "tpc")
                V.tensor_tensor(tpc[:], pc, c2[:], OP.mult)
                e1 = T2B(1, "e1")
                V.tensor_scalar(e1[:], tpc[:], 2.0, None, OP.mult)
                V.tensor_tensor(e1[:], e1[:], c1, OP.add)
                uc2 = T2B(1, "uc2")
                V.tensor_tensor(uc2[:], uu[:], c2[:], OP.mult)
                gam = T2B(1, "gam")
                V.tensor_tensor(gam[:], e1[:], uc2[:], OP.subtract)
                V.tensor_tensor(gam[:], uu[:], gam[:], OP.mult)
                V.tensor_tensor(gam[:], gam[:], lg[:, 0:FD], OP.add)

                # --- outputs ---
                # diag: Yd = c2*(SQ_adf + w3) + e1*Dt + gam
                w3 = T2B(3, "w3")
                SQb3 = SQ[:, 3 * FD:6 * FD].rearrange("p (c f) -> p c f", c=3)
                (G if (PMASK & 2) else V).tensor_tensor(c3(w3), b3(su), SQb3[:, ::-1, :], OP.subtract)
                DD2 = T2B(3, "DD2")
                (G if (PMASK & 16) else V).tensor_tensor(DD2[:], SQ[:, 0:3 * FD], w3[:], OP.add)
                V.tensor_tensor(c3(DD2), c3(DD2), b3(c2[:]), OP.mult)
                m1 = DD2
                u1 = T2B(3, "u1")
                V.tensor_tensor(c3(u1), Dt3[:, ::-1, :], b3(c2[:]), OP.mult)
                (G if (PMASK & 8) else V).tensor_tensor(Dt3, Dt3, b3(e1[:]), OP.mult)
                m2 = Dt
                Yd = T2B(3, "Yd")
                V.tensor_tensor(Yd[:], m1[:], m2[:], OP.add)
                V.tensor_tensor(c3(Yd), c3(Yd), b3(gam[:]), OP.add)

                # offdiag: Yo = bce*(e1 - c2*Dt_rev) + c2*cross
                V.tensor_tensor(c3(u1), b3(e1[:]), c3(u1), OP.subtract)
                V.tensor_tensor(u1[:], bce, u1[:], OP.mult)
                vv = T2B(3, "vv")
                (G if (PMASK & 4) else V).tensor_tensor(c3(vv), c3(cross), b3(c2[:]), OP.mult)
                Yo = T2B(3, "Yo")
                V.tensor_tensor(Yo[:], u1[:], vv[:], OP.add)

                dstd = bass.AP(yout, b * 6 * VPC + j * FD,
                               [[PLANE, P], [VPC, 3], [1, FD]])
                dsto = bass.AP(yout, b * 6 * VPC + 3 * VPC + j * FD,
                               [[PLANE, P], [VPC, 3], [1, FD]])
                nc.sync.dma_start(dstd, c3(Yd))
                nc.sync.dma_start(dsto, c3(Yo))

            tiles = {}
            for k in range(min(AHEAD, NCHUNK)):
                tiles[k] = stageA(k)
            for ci in range(NCHUNK):
                if ci + AHEAD < NCHUNK:
                    tiles[ci + AHEAD] = stageA(ci + AHEAD)
                stageB(ci, tiles.pop(ci))
    nc.finalize()
    return nc


def kernel(x):
    x = np.asarray(x)
    xh = x.reshape(B, 9, NV).astype(np.float16)
    sel = [0, 4, 8, 1, 2, 5]  # a d f b c e
    in_maps = []
    for k in range(NCORE):
        shard = np.ascontiguousarray(xh[:, sel, k * VPC:(k + 1) * VPC])
        in_maps.append({"xin": shard})
    if "nc" not in _CACHE:
        _CACHE["nc"] = build()
    res = run_bass_kernel_spmd(_CACHE["nc"], in_maps, core_ids=list(range(NCORE)))
    out = np.empty((B, 9, NV), np.float32)
    for k in range(NCORE):
        sl = slice(k * VPC, (k + 1) * VPC)
        yk = res.results[k]["yout"].astype(np.float32)
        out[:, 0, sl] = yk[:, 0]
        out[:, 4, sl] = yk[:, 1]
        out[:, 8, sl] = yk[:, 2]
        out[:, 1, sl] = yk[:, 3]
        out[:, 3, sl] = yk[:, 3]
        out[:, 2, sl] = yk[:, 4]
        out[:, 6, sl] = yk[:, 4]
        out[:, 5, sl] = yk[:, 5]
        out[:, 7, sl] = yk[:, 5]
    return out.reshape(x.shape)


# revision 21
# speedup vs baseline: 1.3662x; 1.0440x over previous
"""Trainium2 Bass kernel for per-voxel 3x3 SPD matrix logarithm.

Input  x: (2, 9, 64, 128, 128) fp32, channel c = 3*i+j of symmetric M.
Output Y: same shape, Y = U log(S) U^T per voxel.

Branchless spectral method (fp16 end-to-end on device):
  q = tr(M)/3, D = M - q I, p = sqrt(tr(D^2)/6), r = det(D)/(2 p^3)
  phi = acos(clamp(r))/3 via arctan + sin on ACT
  nodes l1<l2<l3 = q + 2p cos(phi + {-, 0, +}2pi/3), gaps floored at dlo
  divided differences c1, c2 of ln at the nodes (self-consistent: gaps are
  the same f16 values used to build the nodes, so cancellation noise in the
  interpolation coefficients is self-limiting)
  Y = c2 D^2 + e1 D + gam I with e1 = c1 + 2 p cos(phi) c2,
  gam = ln(l1) + (q-l1)(c1 + c2 (q-l2))

fp16 device I/O: host converts input to fp16 (6 unique channels) and expands
the 6 fp16 output channels back to the 9-channel fp32 tensor.

Engines: DVE does fp16 TensorTensor (2x mode) / tensor_scalar (4x mode) bulk,
custom DVE ops for clamp+cube and fp16 reciprocal (BITWISE_NOT seed + NR);
ACT does squares, 1/sqrt (Abs_reciprocal_sqrt), arctan/sin, ln; Pool takes
structural 3-channel products (tau, cross, w3, one output mult).
"""
import math
import numpy as np

import concourse.bacc as bacc
import concourse.tile as tile
import concourse.bass as bass
from concourse import mybir
from concourse.bass_utils import run_bass_kernel_spmd

F32 = mybir.dt.float32
F16 = mybir.dt.float16
OP = mybir.AluOpType
AF = mybir.ActivationFunctionType

B = 2
NV = 64 * 128 * 128
NCORE = 8
VPC = NV // NCORE
P = 128
import os
FD = int(os.environ.get("LOGM_FD", "512"))
NBUF = int(os.environ.get("LOGM_NBUF", "2"))
NBUF_B = int(os.environ.get("LOGM_NBUF_B", "2"))
POOL_A = os.environ.get("LOGM_POOL_A", "0") == "1"
TAU_DVE = os.environ.get("LOGM_TAU_DVE", "1") == "1"
NO_POOL = os.environ.get("LOGM_NO_POOL", "0") == "1"
XBUF = int(os.environ.get("LOGM_XBUF", "4"))
CBUF = int(os.environ.get("LOGM_CBUF", "4"))
AHEAD = int(os.environ.get("LOGM_AHEAD", "1"))
ABUF = int(os.environ.get("LOGM_ABUF", "2"))
PMASK = int(os.environ.get("LOGM_PMASK", "23"))  # 1=cross 2=w3 4=vv 8=m2 16=DD2
CPB = VPC // (P * FD)        # chunks per batch
NCHUNK = B * CPB
PLANE = VPC // P

CL = 0.99999988
S3 = math.sqrt(3.0)
PI6 = math.pi / 6.0
DLO = 0.01                   # eigen-gap floor (fp16 consistency scale)

# ---- runtime-registered custom DVE ops ----
from concourse import dve_ops as _dvo
from concourse.dve_spec import (
    Spec as _Spec, Src0 as _S0, Src1 as _S1, C0 as _C0, C1 as _C1, C2 as _C2,
    maxx as _maxx, minn as _minn, lower as _lower, _has_src1 as _hs1,
    Bin as _Bin, AluOp as _AluOp,
)
from concourse.dve_uop import DveOpSpec as _DveOpSpec


def _register_dve(name, spec):
    if name in _dvo._SUB_OPCODE_FOR_NAME:
        return next(op for op in _dvo.OPS if op.name == name)
    op = _dvo.DveOp(name, spec, subdim=False, uops_sha={})
    _dvo.OPS.append(op)
    _dvo.CUSTOM_DVE_SPECS[name] = spec
    row = _dvo._CUSTOM_DVE_ROW_BASE + len(_dvo.OPS) - 1
    assert row < 0x20
    _dvo._SUB_OPCODE_FOR_NAME[name] = row
    for ver in ("v3", "v4"):
        uops = _lower(spec, ver=ver)
        res = _DveOpSpec(name=name, opcode=row, uops=uops, rd1_en=_hs1(spec))
        op.uops_sha[ver] = res.sha(ver)
    return op


# r = clamp(in0 * in1^3 * s0, s1, imm2)
DETC_CLAMP = _register_dve("LOGM_DETC_CLAMP", _Spec(
    body=_minn(_maxx(_S0 * (_S1 * _S1 * _S1) * _C0, _C1), _C2),
    reference=lambda in0, in1, s0, s1, imm2: np.minimum(
        np.maximum(in0.astype(np.float32) * (in1.astype(np.float32) ** 3) * s0, s1), imm2
    ).astype(np.float32),
))

# fp16-capable approximate reciprocal (fp32 datapath: BITWISE_NOT seed + 2 NR)
_not_x = _Bin(_AluOp.BITWISE_NOT, _S0, _S0)
_y0 = _not_x * _C0
_y1 = _y0 * (_C1 - _S0 * _y0)


def _ref_recip_fast(in0, in1, c0, c1, c2):
    not_x = (~in0.astype(np.float32).view(np.int32)).view(np.float32)
    y0 = not_x * c0
    y1 = y0 * (c1 - in0.astype(np.float32) * y0)
    return y1 * (c2 - in0.astype(np.float32) * y1)


RECIP16 = _register_dve("LOGM_RECIP16", _Spec(
    body=_y1 * (_C2 - _S0 * _y1), reference=_ref_recip_fast))
RECIP_CONSTS = {"s0": -0.23549792, "s1": 2.0017324, "imm2": 2.0}

# Pin Arctan to trig_and_small (it genuinely lives there) so arctan->sin needs
# one table set; pin Square to abs_reciprocal_sqrt_and_small (square is in
# every set) so squares ride the ARS load. Both only steer set CHOICE.
from concourse import hw_specs as _hw
import concourse.bacc as _bacc_mod
_orig_gat = _hw.get_activation_tables


def _patched_gat(arch):
    t = _orig_gat(arch)
    for sname, fns in t.items():
        if sname != "trig_and_small":
            fns.discard(mybir.ActivationFunctionType.Arctan)
        if sname != "abs_reciprocal_sqrt_and_small":
            fns.discard(mybir.ActivationFunctionType.Square)
    return t


_hw.get_activation_tables = _patched_gat
_bacc_mod.get_activation_tables = _patched_gat

_CACHE = {}


def _register_const(nc, val):
    t = nc.alloc_sbuf_tensor(f"const-f32-{val}", [128, 1], F32)
    nc.gpsimd.memset(t.ap(), val)
    nc.const_aps.aps[(F32, float(val))] = t.ap()


def build():
    nc = bacc.Bacc("TRN2")
    _register_const(nc, PI6)
    _register_const(nc, PI6 + math.pi / 2.0)
    _register_const(nc, 1.0)
    nc.all_engine_barrier()
    xin = nc.dram_tensor("xin", [B, 6, VPC], F16, kind="ExternalInput")
    yout = nc.dram_tensor("yout", [B, 6, VPC], F16, kind="ExternalOutput")

    V, S = nc.vector, nc.scalar
    G = nc.vector if NO_POOL else nc.gpsimd

    with tile.TileContext(nc) as tc:
        with tc.tile_pool(name="mp", bufs=1) as pool:

            def T2(units, name, dt=F16, bufs=None):
                bufs = NBUF if bufs is None else bufs
                return pool.tile([P, units * FD], dt, name=name, tag=name, bufs=bufs)

            def b3(ap_fd):
                return ap_fd.unsqueeze(1).broadcast_to((P, 3, FD))

            def b2(ap_fd):
                return ap_fd.unsqueeze(1).broadcast_to((P, 2, FD))

            def c3(t):
                return t[:].rearrange("p (c f) -> p c f", c=3)

            def stageA(ci):
                b, j = divmod(ci, CPB)
                t = {}
                xin_t = T2(6, "xin", bufs=(XBUF or NBUF))
                t["xin"] = xin_t
                src1 = bass.AP(xin, b * 6 * VPC + j * FD,
                               [[PLANE, P], [VPC, 3], [1, FD]])
                src2 = bass.AP(xin, b * 6 * VPC + 3 * VPC + j * FD,
                               [[PLANE, P], [VPC, 3], [1, FD]])
                nc.sync.dma_start(xin_t[:, 0:3 * FD].rearrange("p (c f) -> p c f", c=3), src1)
                nc.sync.dma_start(xin_t[:, 3 * FD:6 * FD].rearrange("p (c f) -> p c f", c=3), src2)
                a_ = xin_t[:, 0 * FD:1 * FD]
                d_ = xin_t[:, 1 * FD:2 * FD]
                f_ = xin_t[:, 2 * FD:3 * FD]
                b_ = xin_t[:, 3 * FD:4 * FD]
                c_ = xin_t[:, 4 * FD:5 * FD]
                e_ = xin_t[:, 5 * FD:6 * FD]
                adf = xin_t[:, 0:3 * FD].rearrange("p (c f) -> p c f", c=3)
                bce = xin_t[:, 3 * FD:6 * FD]

                s1 = T2(1, "s1", bufs=ABUF)
                E1 = G if POOL_A else V
                E1.tensor_tensor(s1[:], a_, d_, OP.add)
                E1.tensor_tensor(s1[:], s1[:], f_, OP.add)
                q = T2(1, "q", bufs=(CBUF or NBUF))
                V.tensor_scalar(q[:], s1[:], 1.0 / 3.0, None, OP.mult)
                t["q"] = q

                Dt = T2(3, "Dt", bufs=(CBUF or NBUF))
                V.tensor_tensor(c3(Dt), adf, b3(q[:]), OP.subtract)
                t["Dt"] = Dt
                aa = Dt[:, 0:FD]
                dd = Dt[:, FD:2 * FD]

                SQ = T2(6, "SQ", bufs=(CBUF or NBUF))
                S.activation(SQ[:, 0:3 * FD], Dt[:], AF.Square)
                sq_i = S.activation(SQ[:, 3 * FD:6 * FD], bce, AF.Square)
                t["SQ"] = SQ
                t["sq_inst"] = sq_i
                SQb3 = SQ[:, 3 * FD:6 * FD].rearrange("p (c f) -> p c f", c=3)

                stsu = T2(2, "stsu", bufs=(CBUF or NBUF))
                pA = SQ[:].rearrange("p (c f) -> p c f", c=6)
                st2 = stsu[:].rearrange("p (c f) -> p c f", c=2)
                V.tensor_tensor(st2, pA[:, 0:6:3, :], pA[:, 1:6:3, :], OP.add)
                V.tensor_tensor(st2, st2, pA[:, 2:6:3, :], OP.add)
                t["stsu"] = stsu
                p2s6 = T2(1, "p2s6", bufs=(CBUF or NBUF))
                V.scalar_tensor_tensor(p2s6[:], stsu[:, FD:2 * FD], 2.0,
                                       stsu[:, 0:FD], OP.mult, OP.add)
                t["p2s6"] = p2s6

                # det(D): tau_i = Dt_i * SQ_bce_rev_i on Pool
                tau = T2(3, "tau", bufs=ABUF)
                (V if TAU_DVE else G).tensor_tensor(c3(tau), c3(Dt), SQb3[:, ::-1, :], OP.mult)
                dets = T2(1, "dets", bufs=ABUF)
                EA = G if POOL_A else V
                EA.tensor_tensor(dets[:], tau[:, 0:FD], tau[:, FD:2 * FD], OP.add)
                EA.tensor_tensor(dets[:], dets[:], tau[:, 2 * FD:3 * FD], OP.add)
                ad3 = T2(1, "ad3", bufs=ABUF)
                EA.tensor_tensor(ad3[:], aa, dd, OP.mult)
                EA.tensor_tensor(ad3[:], ad3[:], Dt[:, 2 * FD:3 * FD], OP.mult)
                det = T2(1, "det", bufs=ABUF)
                V.tensor_tensor(det[:], ad3[:], dets[:], OP.subtract)

                cross = T2(3, "cross", bufs=(CBUF or NBUF))
                cb_ap = xin_t[:, 3 * FD:5 * FD].rearrange("p (c f) -> p c f", c=2)[:, ::-1, :]
                EC = G if (PMASK & 1) else V
                EC.tensor_tensor(cross[:, 0:2 * FD].rearrange("p (c f) -> p c f", c=2),
                                cb_ap, b2(e_), OP.mult)
                EC.tensor_tensor(cross[:, 2 * FD:3 * FD], b_, c_, OP.mult)
                t["cross"] = cross
                bcep = T2(1, "bcep", bufs=(CBUF or NBUF))
                V.tensor_tensor(bcep[:], cross[:, 2 * FD:3 * FD], e_, OP.mult)
                t["bcep"] = bcep
                det32 = T2(1, "det32", bufs=(CBUF or NBUF))
                V.tensor_scalar(det32[:], bcep[:], 2.0, None, OP.mult)
                V.tensor_tensor(det32[:], det32[:], det[:], OP.add)
                t["det32"] = det32
                return t

            def stageB(ci, t):
                def T2B(units, name, dt=F16):
                    return pool.tile([P, units * FD], dt, name=name, tag=name,
                                     bufs=NBUF_B)
                b, j = divmod(ci, CPB)
                xin_t = t["xin"]; Dt = t["Dt"]; SQ = t["SQ"]
                q = t["q"]; p2s6 = t["p2s6"]; cross = t["cross"]
                stsu = t["stsu"]; det32 = t["det32"]
                Dt3 = c3(Dt)
                bce = xin_t[:, 3 * FD:6 * FD]
                su = stsu[:, FD:2 * FD]

                # --- scalar chain ---
                ip = T2B(1, "ip")
                ip_i = S.activation(ip[:], p2s6[:], AF.Abs_reciprocal_sqrt, scale=1.0 / 6.0)
                nxt = t.get("next_sq")
                if nxt is not None:
                    pass
                pt16 = T2B(1, "pt16")
                V.tensor_tensor(pt16[:], p2s6[:], ip[:], OP.mult)
                V.tensor_scalar(pt16[:], pt16[:], 1.0 / 6.0, None, OP.mult)
                rr = T2B(1, "rr")
                V._custom_dve(DETC_CLAMP, out=rr[:], in0=det32[:], in1=ip[:],
                              s0=0.5, s1=-CL, imm2=CL)
                r2 = T2B(1, "r2")
                S.activation(r2[:], rr[:], AF.Square)
                isq = T2B(1, "isq")
                S.activation(isq[:], r2[:], AF.Abs_reciprocal_sqrt, scale=-1.0, bias=1.0)
                tq = T2B(1, "tq")
                V.tensor_tensor(tq[:], rr[:], isq[:], OP.mult)
                at = T2B(1, "at", dt=F32)
                S.activation(at[:], tq[:], AF.Arctan)
                sfcf = T2B(2, "sfcf")
                S.activation(sfcf[:, 0:FD], at[:], AF.Sin, scale=-1.0 / 3.0, bias=PI6)
                S.activation(sfcf[:, FD:2 * FD], at[:], AF.Sin, scale=-1.0 / 3.0,
                             bias=PI6 + math.pi / 2.0)

                pcps = T2B(2, "pcps")
                V.tensor_tensor(pcps[:].rearrange("p (c f) -> p c f", c=2),
                                b2(pt16[:]), sfcf[:].rearrange("p (c f) -> p c f", c=2),
                                OP.mult)
                ps = pcps[:, 0:FD]; pc = pcps[:, FD:2 * FD]
                tsp = T2B(1, "tsp")
                V.tensor_scalar(tsp[:], ps, S3, None, OP.mult)
                uu = T2B(1, "uu")
                V.tensor_tensor(uu[:], tsp[:], pc, OP.add)

                LD = T2B(6, "LD")  # [l1|l2|l3|d12|d23|d13]
                l1 = LD[:, 0:FD]; l2 = LD[:, FD:2 * FD]; l3 = LD[:, 2 * FD:3 * FD]
                d23 = LD[:, 4 * FD:5 * FD]
                V.tensor_tensor(l1, q[:], uu[:], OP.subtract)
                V.tensor_scalar(LD[:, 3 * FD:4 * FD], ps, 2.0 * S3, DLO, OP.mult, OP.max)
                tt3 = T2B(1, "tt3")
                V.tensor_scalar(tt3[:], pc, 3.0, None, OP.mult)
                V.tensor_tensor(d23, tt3[:], tsp[:], OP.subtract)
                V.tensor_scalar(d23, d23, 1.0, DLO, OP.mult, OP.max)
                LD6 = LD[:].rearrange("p (c f) -> p c f", c=6)
                # (l2, d13) = (l1, d12) + (d12, d23)
                V.tensor_tensor(LD6[:, 1:6:4, :], LD6[:, 0:4:3, :], LD6[:, 3:5:1, :], OP.add)
                V.tensor_tensor(l3, l2, d23, OP.add)

                ii = T2B(3, "ii")
                V._custom_dve(RECIP16, out=ii[:], in0=LD[:, 3 * FD:6 * FD], **RECIP_CONSTS)
                lg = T2B(3, "lg")
                S.activation(lg[:], LD[:, 0:3 * FD], AF.Ln)
                gg = T2B(2, "gg")
                V.tensor_tensor(gg[:], lg[:, FD:3 * FD], lg[:, 0:2 * FD], OP.subtract)
                c1f = T2B(2, "c1f")
                V.tensor_tensor(c1f[:], gg[:], ii[:, 0:2 * FD], OP.mult)
                c1 = c1f[:, 0:FD]
                c2 = T2B(1, "c2")
                V.tensor_tensor(c2[:], c1f[:, FD:2 * FD], c1, OP.subtract)
                V.tensor_tensor(c2[:], c2[:], ii[:, 2 * FD:3 * FD], OP.mult)

                # tail: e1 = c1 + 2 pc c2 ; gam = lg1 + uu*(e1 - uu*c2)
                tpc = T2B(1, "tpc")
                V.tensor_tensor(tpc[:], pc, c2[:], OP.mult)
                e1 = T2B(1, "e1")
                V.tensor_scalar(e1[:], tpc[:], 2.0, None, OP.mult)
                V.tensor_tensor(e1[:], e1[:], c1, OP.add)
                uc2 = T2B(1, "uc2")
                V.tensor_tensor(uc2[:], uu[:], c2[:], OP.mult)
                gam = T2B(1, "gam")
                V.tensor_tensor(gam[:], e1[:], uc2[:], OP.subtract)
                V.tensor_tensor(gam[:], uu[:], gam[:], OP.mult)
                V.tensor_tensor(gam[:], gam[:], lg[:, 0:FD], OP.add)

                # --- outputs ---
                # diag: Yd = c2*(SQ_adf + w3) + e1*Dt + gam
                w3 = T2B(3, "w3")
                SQb3 = SQ[:, 3 * FD:6 * FD].rearrange("p (c f) -> p c f", c=3)
                (G if (PMASK & 2) else V).tensor_tensor(c3(w3), b3(su), SQb3[:, ::-1, :], OP.subtract)
                DD2 = T2B(3, "DD2")
                (G if (PMASK & 16) else V).tensor_tensor(DD2[:], SQ[:, 0:3 * FD], w3[:], OP.add)
                V.tensor_tensor(c3(DD2), c3(DD2), b3(c2[:]), OP.mult)
                m1 = DD2
                u1 = T2B(3, "u1")
                V.tensor_tensor(c3(u1), Dt3[:, ::-1, :], b3(c2[:]), OP.mult)
                (G if (PMASK & 8) else V).tensor_tensor(Dt3, Dt3, b3(e1[:]), OP.mult)
                m2 = Dt
                V.tensor_tensor(m1[:], m1[:], m2[:], OP.add)
                V.tensor_tensor(c3(m1), c3(m1), b3(gam[:]), OP.add)
                Yd = m1

                # offdiag: Yo = bce*(e1 - c2*Dt_rev) + c2*cross
                V.tensor_tensor(c3(u1), b3(e1[:]), c3(u1), OP.subtract)
                V.tensor_tensor(u1[:], bce, u1[:], OP.mult)
                vv = T2B(3, "vv")
                (G if (PMASK & 4) else V).tensor_tensor(c3(vv), c3(cross), b3(c2[:]), OP.mult)
                V.tensor_tensor(u1[:], u1[:], vv[:], OP.add)
                Yo = u1

                dstd = bass.AP(yout, b * 6 * VPC + j * FD,
                               [[PLANE, P], [VPC, 3], [1, FD]])
                dsto = bass.AP(yout, b * 6 * VPC + 3 * VPC + j * FD,
                               [[PLANE, P], [VPC, 3], [1, FD]])
                nc.sync.dma_start(dstd, c3(Yd))
                nc.sync.dma_start(dsto, c3(Yo))

            tiles = {}
            for k in range(min(AHEAD, NCHUNK)):
                tiles[k] = stageA(k)
            for ci in range(NCHUNK):
                if ci + AHEAD < NCHUNK:
                    tiles[ci + AHEAD] = stageA(ci + AHEAD)
                stageB(ci, tiles.pop(ci))
    nc.finalize()
    return nc


def kernel(x):
    x = np.asarray(x)
    xh = x.reshape(B, 9, NV).astype(np.float16)
    sel = [0, 4, 8, 1, 2, 5]  # a d f b c e
    in_maps = []
    for k in range(NCORE):
        shard = np.ascontiguousarray(xh[:, sel, k * VPC:(k + 1) * VPC])
        in_maps.append({"xin": shard})
    if "nc" not in _CACHE:
        _CACHE["nc"] = build()
    res = run_bass_kernel_spmd(_CACHE["nc"], in_maps, core_ids=list(range(NCORE)))
    out = np.empty((B, 9, NV), np.float32)
    for k in range(NCORE):
        sl = slice(k * VPC, (k + 1) * VPC)
        yk = res.results[k]["yout"].astype(np.float32)
        out[:, 0, sl] = yk[:, 0]
        out[:, 4, sl] = yk[:, 1]
        out[:, 8, sl] = yk[:, 2]
        out[:, 1, sl] = yk[:, 3]
        out[:, 3, sl] = yk[:, 3]
        out[:, 2, sl] = yk[:, 4]
        out[:, 6, sl] = yk[:, 4]
        out[:, 5, sl] = yk[:, 5]
        out[:, 7, sl] = yk[:, 5]
    return out.reshape(x.shape)


# revision 22
# speedup vs baseline: 1.3763x; 1.0074x over previous
"""Trainium2 Bass kernel for per-voxel 3x3 SPD matrix logarithm.

Input  x: (2, 9, 64, 128, 128) fp32, channel c = 3*i+j of symmetric M.
Output Y: same shape, Y = U log(S) U^T per voxel.

Branchless spectral method (fp16 end-to-end on device):
  q = tr(M)/3, D = M - q I, p = sqrt(tr(D^2)/6), r = det(D)/(2 p^3)
  phi = acos(clamp(r))/3 via arctan + sin on ACT
  nodes l1<l2<l3 = q + 2p cos(phi + {-, 0, +}2pi/3), gaps floored at dlo
  divided differences c1, c2 of ln at the nodes (self-consistent: gaps are
  the same f16 values used to build the nodes, so cancellation noise in the
  interpolation coefficients is self-limiting)
  Y = c2 D^2 + e1 D + gam I with e1 = c1 + 2 p cos(phi) c2,
  gam = ln(l1) + (q-l1)(c1 + c2 (q-l2))

fp16 device I/O: host converts input to fp16 (6 unique channels) and expands
the 6 fp16 output channels back to the 9-channel fp32 tensor.

Engines: DVE does fp16 TensorTensor (2x mode) / tensor_scalar (4x mode) bulk,
custom DVE ops for clamp+cube and fp16 reciprocal (BITWISE_NOT seed + NR);
ACT does squares, 1/sqrt (Abs_reciprocal_sqrt), arctan/sin, ln; Pool takes
structural 3-channel products (tau, cross, w3, one output mult).
"""
import math
import numpy as np

import concourse.bacc as bacc
import concourse.tile as tile
import concourse.bass as bass
from concourse import mybir
from concourse.bass_utils import run_bass_kernel_spmd

F32 = mybir.dt.float32
F16 = mybir.dt.float16
OP = mybir.AluOpType
AF = mybir.ActivationFunctionType

B = 2
NV = 64 * 128 * 128
NCORE = 8
VPC = NV // NCORE
P = 128
import os
FD = int(os.environ.get("LOGM_FD", "512"))
NBUF = int(os.environ.get("LOGM_NBUF", "2"))
NBUF_B = int(os.environ.get("LOGM_NBUF_B", "2"))
POOL_A = os.environ.get("LOGM_POOL_A", "0") == "1"
TAU_DVE = os.environ.get("LOGM_TAU_DVE", "1") == "1"
NO_POOL = os.environ.get("LOGM_NO_POOL", "0") == "1"
XBUF = int(os.environ.get("LOGM_XBUF", "4"))
CBUF = int(os.environ.get("LOGM_CBUF", "4"))
AHEAD = int(os.environ.get("LOGM_AHEAD", "1"))
ABUF = int(os.environ.get("LOGM_ABUF", "2"))
SOFF = int(os.environ.get("LOGM_SOFF", "0"))  # 1=q16 2=tt3 4=e1scale on ACT
PMASK = int(os.environ.get("LOGM_PMASK", "23"))  # 1=cross 2=w3 4=vv 8=m2 16=DD2
CPB = VPC // (P * FD)        # chunks per batch
NCHUNK = B * CPB
PLANE = VPC // P

CL = 0.99999988
S3 = math.sqrt(3.0)
PI6 = math.pi / 6.0
DLO = 0.01                   # eigen-gap floor (fp16 consistency scale)

# ---- runtime-registered custom DVE ops ----
from concourse import dve_ops as _dvo
from concourse.dve_spec import (
    Spec as _Spec, Src0 as _S0, Src1 as _S1, C0 as _C0, C1 as _C1, C2 as _C2,
    maxx as _maxx, minn as _minn, lower as _lower, _has_src1 as _hs1,
    Bin as _Bin, AluOp as _AluOp,
)
from concourse.dve_uop import DveOpSpec as _DveOpSpec


def _register_dve(name, spec):
    if name in _dvo._SUB_OPCODE_FOR_NAME:
        return next(op for op in _dvo.OPS if op.name == name)
    op = _dvo.DveOp(name, spec, subdim=False, uops_sha={})
    _dvo.OPS.append(op)
    _dvo.CUSTOM_DVE_SPECS[name] = spec
    row = _dvo._CUSTOM_DVE_ROW_BASE + len(_dvo.OPS) - 1
    assert row < 0x20
    _dvo._SUB_OPCODE_FOR_NAME[name] = row
    for ver in ("v3", "v4"):
        uops = _lower(spec, ver=ver)
        res = _DveOpSpec(name=name, opcode=row, uops=uops, rd1_en=_hs1(spec))
        op.uops_sha[ver] = res.sha(ver)
    return op


# r = clamp(in0 * in1^3 * s0, s1, imm2)
DETC_CLAMP = _register_dve("LOGM_DETC_CLAMP", _Spec(
    body=_minn(_maxx(_S0 * (_S1 * _S1 * _S1) * _C0, _C1), _C2),
    reference=lambda in0, in1, s0, s1, imm2: np.minimum(
        np.maximum(in0.astype(np.float32) * (in1.astype(np.float32) ** 3) * s0, s1), imm2
    ).astype(np.float32),
))

# fp16-capable approximate reciprocal (fp32 datapath: BITWISE_NOT seed + 2 NR)
_not_x = _Bin(_AluOp.BITWISE_NOT, _S0, _S0)
_y0 = _not_x * _C0
_y1 = _y0 * (_C1 - _S0 * _y0)


def _ref_recip_fast(in0, in1, c0, c1, c2):
    not_x = (~in0.astype(np.float32).view(np.int32)).view(np.float32)
    y0 = not_x * c0
    y1 = y0 * (c1 - in0.astype(np.float32) * y0)
    return y1 * (c2 - in0.astype(np.float32) * y1)


RECIP16 = _register_dve("LOGM_RECIP16", _Spec(
    body=_y1 * (_C2 - _S0 * _y1), reference=_ref_recip_fast))
RECIP_CONSTS = {"s0": -0.23549792, "s1": 2.0017324, "imm2": 2.0}

# Pin Arctan to trig_and_small (it genuinely lives there) so arctan->sin needs
# one table set; pin Square to abs_reciprocal_sqrt_and_small (square is in
# every set) so squares ride the ARS load. Both only steer set CHOICE.
from concourse import hw_specs as _hw
import concourse.bacc as _bacc_mod
_orig_gat = _hw.get_activation_tables


def _patched_gat(arch):
    t = _orig_gat(arch)
    for sname, fns in t.items():
        if sname != "trig_and_small":
            fns.discard(mybir.ActivationFunctionType.Arctan)
        if sname != "abs_reciprocal_sqrt_and_small":
            fns.discard(mybir.ActivationFunctionType.Square)
    return t


_hw.get_activation_tables = _patched_gat
_bacc_mod.get_activation_tables = _patched_gat

_CACHE = {}


def _register_const(nc, val):
    t = nc.alloc_sbuf_tensor(f"const-f32-{val}", [128, 1], F32)
    nc.gpsimd.memset(t.ap(), val)
    nc.const_aps.aps[(F32, float(val))] = t.ap()


def build():
    nc = bacc.Bacc("TRN2")
    _register_const(nc, PI6)
    _register_const(nc, PI6 + math.pi / 2.0)
    _register_const(nc, 1.0)
    nc.all_engine_barrier()
    xin = nc.dram_tensor("xin", [B, 6, VPC], F16, kind="ExternalInput")
    yout = nc.dram_tensor("yout", [B, 6, VPC], F16, kind="ExternalOutput")

    V, S = nc.vector, nc.scalar
    G = nc.vector if NO_POOL else nc.gpsimd

    with tile.TileContext(nc) as tc:
        with tc.tile_pool(name="mp", bufs=1) as pool:

            def T2(units, name, dt=F16, bufs=None):
                bufs = NBUF if bufs is None else bufs
                return pool.tile([P, units * FD], dt, name=name, tag=name, bufs=bufs)

            def b3(ap_fd):
                return ap_fd.unsqueeze(1).broadcast_to((P, 3, FD))

            def b2(ap_fd):
                return ap_fd.unsqueeze(1).broadcast_to((P, 2, FD))

            def c3(t):
                return t[:].rearrange("p (c f) -> p c f", c=3)

            def stageA(ci):
                b, j = divmod(ci, CPB)
                t = {}
                xin_t = T2(6, "xin", bufs=(XBUF or NBUF))
                t["xin"] = xin_t
                src1 = bass.AP(xin, b * 6 * VPC + j * FD,
                               [[PLANE, P], [VPC, 3], [1, FD]])
                src2 = bass.AP(xin, b * 6 * VPC + 3 * VPC + j * FD,
                               [[PLANE, P], [VPC, 3], [1, FD]])
                nc.sync.dma_start(xin_t[:, 0:3 * FD].rearrange("p (c f) -> p c f", c=3), src1)
                nc.sync.dma_start(xin_t[:, 3 * FD:6 * FD].rearrange("p (c f) -> p c f", c=3), src2)
                a_ = xin_t[:, 0 * FD:1 * FD]
                d_ = xin_t[:, 1 * FD:2 * FD]
                f_ = xin_t[:, 2 * FD:3 * FD]
                b_ = xin_t[:, 3 * FD:4 * FD]
                c_ = xin_t[:, 4 * FD:5 * FD]
                e_ = xin_t[:, 5 * FD:6 * FD]
                adf = xin_t[:, 0:3 * FD].rearrange("p (c f) -> p c f", c=3)
                bce = xin_t[:, 3 * FD:6 * FD]

                s1 = T2(1, "s1", bufs=ABUF)
                E1 = G if POOL_A else V
                E1.tensor_tensor(s1[:], a_, d_, OP.add)
                E1.tensor_tensor(s1[:], s1[:], f_, OP.add)
                q = T2(1, "q", bufs=(CBUF or NBUF))
                if SOFF & 1:
                    S.activation(q[:], s1[:], AF.Copy, scale=1.0 / 3.0)
                else:
                    V.tensor_scalar(q[:], s1[:], 1.0 / 3.0, None, OP.mult)
                t["q"] = q

                Dt = T2(3, "Dt", bufs=(CBUF or NBUF))
                V.tensor_tensor(c3(Dt), adf, b3(q[:]), OP.subtract)
                t["Dt"] = Dt
                aa = Dt[:, 0:FD]
                dd = Dt[:, FD:2 * FD]

                SQ = T2(6, "SQ", bufs=(CBUF or NBUF))
                S.activation(SQ[:, 0:3 * FD], Dt[:], AF.Square)
                sq_i = S.activation(SQ[:, 3 * FD:6 * FD], bce, AF.Square)
                t["SQ"] = SQ
                t["sq_inst"] = sq_i
                SQb3 = SQ[:, 3 * FD:6 * FD].rearrange("p (c f) -> p c f", c=3)

                stsu = T2(2, "stsu", bufs=(CBUF or NBUF))
                pA = SQ[:].rearrange("p (c f) -> p c f", c=6)
                st2 = stsu[:].rearrange("p (c f) -> p c f", c=2)
                V.tensor_tensor(st2, pA[:, 0:6:3, :], pA[:, 1:6:3, :], OP.add)
                V.tensor_tensor(st2, st2, pA[:, 2:6:3, :], OP.add)
                t["stsu"] = stsu
                p2s6 = T2(1, "p2s6", bufs=(CBUF or NBUF))
                V.scalar_tensor_tensor(p2s6[:], stsu[:, FD:2 * FD], 2.0,
                                       stsu[:, 0:FD], OP.mult, OP.add)
                t["p2s6"] = p2s6

                # det(D): tau_i = Dt_i * SQ_bce_rev_i on Pool
                tau = T2(3, "tau", bufs=ABUF)
                (V if TAU_DVE else G).tensor_tensor(c3(tau), c3(Dt), SQb3[:, ::-1, :], OP.mult)
                dets = T2(1, "dets", bufs=ABUF)
                EA = G if POOL_A else V
                EA.tensor_tensor(dets[:], tau[:, 0:FD], tau[:, FD:2 * FD], OP.add)
                EA.tensor_tensor(dets[:], dets[:], tau[:, 2 * FD:3 * FD], OP.add)
                ad3 = T2(1, "ad3", bufs=ABUF)
                EA.tensor_tensor(ad3[:], aa, dd, OP.mult)
                EA.tensor_tensor(ad3[:], ad3[:], Dt[:, 2 * FD:3 * FD], OP.mult)
                det = T2(1, "det", bufs=ABUF)
                V.tensor_tensor(det[:], ad3[:], dets[:], OP.subtract)

                cross = T2(3, "cross", bufs=(CBUF or NBUF))
                cb_ap = xin_t[:, 3 * FD:5 * FD].rearrange("p (c f) -> p c f", c=2)[:, ::-1, :]
                EC = G if (PMASK & 1) else V
                EC.tensor_tensor(cross[:, 0:2 * FD].rearrange("p (c f) -> p c f", c=2),
                                cb_ap, b2(e_), OP.mult)
                EC.tensor_tensor(cross[:, 2 * FD:3 * FD], b_, c_, OP.mult)
                t["cross"] = cross
                bcep = T2(1, "bcep", bufs=(CBUF or NBUF))
                V.tensor_tensor(bcep[:], cross[:, 2 * FD:3 * FD], e_, OP.mult)
                t["bcep"] = bcep
                det32 = T2(1, "det32", bufs=(CBUF or NBUF))
                V.tensor_scalar(det32[:], bcep[:], 2.0, None, OP.mult)
                V.tensor_tensor(det32[:], det32[:], det[:], OP.add)
                t["det32"] = det32
                return t

            def stageB(ci, t):
                def T2B(units, name, dt=F16):
                    return pool.tile([P, units * FD], dt, name=name, tag=name,
                                     bufs=NBUF_B)
                b, j = divmod(ci, CPB)
                xin_t = t["xin"]; Dt = t["Dt"]; SQ = t["SQ"]
                q = t["q"]; p2s6 = t["p2s6"]; cross = t["cross"]
                stsu = t["stsu"]; det32 = t["det32"]
                Dt3 = c3(Dt)
                bce = xin_t[:, 3 * FD:6 * FD]
                su = stsu[:, FD:2 * FD]

                # --- scalar chain ---
                ip = T2B(1, "ip")
                S.activation(ip[:], p2s6[:], AF.Abs_reciprocal_sqrt, scale=6.0)
                pt16 = T2B(1, "pt16")
                V.tensor_tensor(pt16[:], p2s6[:], ip[:], OP.mult)
                rr = T2B(1, "rr")
                V._custom_dve(DETC_CLAMP, out=rr[:], in0=det32[:], in1=ip[:],
                              s0=108.0, s1=-CL, imm2=CL)
                r2 = T2B(1, "r2")
                S.activation(r2[:], rr[:], AF.Square)
                isq = T2B(1, "isq")
                S.activation(isq[:], r2[:], AF.Abs_reciprocal_sqrt, scale=-1.0, bias=1.0)
                tq = T2B(1, "tq")
                V.tensor_tensor(tq[:], rr[:], isq[:], OP.mult)
                at = T2B(1, "at", dt=F32)
                S.activation(at[:], tq[:], AF.Arctan)
                sfcf = T2B(2, "sfcf")
                S.activation(sfcf[:, 0:FD], at[:], AF.Sin, scale=-1.0 / 3.0, bias=PI6)
                S.activation(sfcf[:, FD:2 * FD], at[:], AF.Sin, scale=-1.0 / 3.0,
                             bias=PI6 + math.pi / 2.0)

                pcps = T2B(2, "pcps")
                V.tensor_tensor(pcps[:].rearrange("p (c f) -> p c f", c=2),
                                b2(pt16[:]), sfcf[:].rearrange("p (c f) -> p c f", c=2),
                                OP.mult)
                ps = pcps[:, 0:FD]; pc = pcps[:, FD:2 * FD]
                tsp = T2B(1, "tsp")
                V.tensor_scalar(tsp[:], ps, S3, None, OP.mult)
                uu = T2B(1, "uu")
                V.tensor_tensor(uu[:], tsp[:], pc, OP.add)

                LD = T2B(6, "LD")  # [l1|l2|l3|d12|d23|d13]
                l1 = LD[:, 0:FD]; l2 = LD[:, FD:2 * FD]; l3 = LD[:, 2 * FD:3 * FD]
                d23 = LD[:, 4 * FD:5 * FD]
                V.tensor_tensor(l1, q[:], uu[:], OP.subtract)
                V.tensor_scalar(LD[:, 3 * FD:4 * FD], ps, 2.0 * S3, DLO, OP.mult, OP.max)
                tt3 = T2B(1, "tt3")
                if SOFF & 2:
                    S.activation(tt3[:], pc, AF.Copy, scale=3.0)
                else:
                    V.tensor_scalar(tt3[:], pc, 3.0, None, OP.mult)
                V.tensor_tensor(d23, tt3[:], tsp[:], OP.subtract)
                V.tensor_scalar(d23, d23, 1.0, DLO, OP.mult, OP.max)
                LD6 = LD[:].rearrange("p (c f) -> p c f", c=6)
                # (l2, d13) = (l1, d12) + (d12, d23)
                V.tensor_tensor(LD6[:, 1:6:4, :], LD6[:, 0:4:3, :], LD6[:, 3:5:1, :], OP.add)
                V.tensor_tensor(l3, l2, d23, OP.add)

                ii = T2B(3, "ii")
                V._custom_dve(RECIP16, out=ii[:], in0=LD[:, 3 * FD:6 * FD], **RECIP_CONSTS)
                lg = T2B(3, "lg")
                S.activation(lg[:], LD[:, 0:3 * FD], AF.Ln)
                gg = T2B(2, "gg")
                V.tensor_tensor(gg[:], lg[:, FD:3 * FD], lg[:, 0:2 * FD], OP.subtract)
                c1f = T2B(2, "c1f")
                V.tensor_tensor(c1f[:], gg[:], ii[:, 0:2 * FD], OP.mult)
                c1 = c1f[:, 0:FD]
                c2 = T2B(1, "c2")
                V.tensor_tensor(c2[:], c1f[:, FD:2 * FD], c1, OP.subtract)
                V.tensor_tensor(c2[:], c2[:], ii[:, 2 * FD:3 * FD], OP.mult)

                # tail: e1 = c1 + 2 pc c2 ; gam = lg1 + uu*(e1 - uu*c2)
                tpc = T2B(1, "tpc")
                V.tensor_tensor(tpc[:], pc, c2[:], OP.mult)
                e1 = T2B(1, "e1")
                if SOFF & 4:
                    S.activation(e1[:], tpc[:], AF.Copy, scale=2.0)
                else:
                    V.tensor_scalar(e1[:], tpc[:], 2.0, None, OP.mult)
                V.tensor_tensor(e1[:], e1[:], c1, OP.add)
                uc2 = T2B(1, "uc2")
                V.tensor_tensor(uc2[:], uu[:], c2[:], OP.mult)
                gam = T2B(1, "gam")
                V.tensor_tensor(gam[:], e1[:], uc2[:], OP.subtract)
                V.tensor_tensor(gam[:], uu[:], gam[:], OP.mult)
                V.tensor_tensor(gam[:], gam[:], lg[:, 0:FD], OP.add)

                # --- outputs ---
                # diag: Yd = c2*(SQ_adf + w3) + e1*Dt + gam
                w3 = T2B(3, "w3")
                SQb3 = SQ[:, 3 * FD:6 * FD].rearrange("p (c f) -> p c f", c=3)
                (G if (PMASK & 2) else V).tensor_tensor(c3(w3), b3(su), SQb3[:, ::-1, :], OP.subtract)
                DD2 = T2B(3, "DD2")
                (G if (PMASK & 16) else V).tensor_tensor(DD2[:], SQ[:, 0:3 * FD], w3[:], OP.add)
                V.tensor_tensor(c3(DD2), c3(DD2), b3(c2[:]), OP.mult)
                m1 = DD2
                u1 = T2B(3, "u1")
                V.tensor_tensor(c3(u1), Dt3[:, ::-1, :], b3(c2[:]), OP.mult)
                (G if (PMASK & 8) else V).tensor_tensor(Dt3, Dt3, b3(e1[:]), OP.mult)
                m2 = Dt
                V.tensor_tensor(m1[:], m1[:], m2[:], OP.add)
                V.tensor_tensor(c3(m1), c3(m1), b3(gam[:]), OP.add)
                Yd = m1

                # offdiag: Yo = bce*(e1 - c2*Dt_rev) + c2*cross
                V.tensor_tensor(c3(u1), b3(e1[:]), c3(u1), OP.subtract)
                V.tensor_tensor(u1[:], bce, u1[:], OP.mult)
                vv = T2B(3, "vv")
                (G if (PMASK & 4) else V).tensor_tensor(c3(vv), c3(cross), b3(c2[:]), OP.mult)
                V.tensor_tensor(u1[:], u1[:], vv[:], OP.add)
                Yo = u1

                dstd = bass.AP(yout, b * 6 * VPC + j * FD,
                               [[PLANE, P], [VPC, 3], [1, FD]])
                dsto = bass.AP(yout, b * 6 * VPC + 3 * VPC + j * FD,
                               [[PLANE, P], [VPC, 3], [1, FD]])
                nc.sync.dma_start(dstd, c3(Yd))
                nc.sync.dma_start(dsto, c3(Yo))

            tiles = {}
            for k in range(min(AHEAD, NCHUNK)):
                tiles[k] = stageA(k)
            for ci in range(NCHUNK):
                if ci + AHEAD < NCHUNK:
                    tiles[ci + AHEAD] = stageA(ci + AHEAD)
                stageB(ci, tiles.pop(ci))
    nc.finalize()
    return nc


def kernel(x):
    x = np.asarray(x)
    xh = x.reshape(B, 9, NV).astype(np.float16)
    sel = [0, 4, 8, 1, 2, 5]  # a d f b c e
    in_maps = []
    for k in range(NCORE):
        shard = np.ascontiguousarray(xh[:, sel, k * VPC:(k + 1) * VPC])
        in_maps.append({"xin": shard})
    if "nc" not in _CACHE:
        _CACHE["nc"] = build()
    res = run_bass_kernel_spmd(_CACHE["nc"], in_maps, core_ids=list(range(NCORE)))
    out = np.empty((B, 9, NV), np.float32)
    for k in range(NCORE):
        sl = slice(k * VPC, (k + 1) * VPC)
        yk = res.results[k]["yout"].astype(np.float32)
        out[:, 0, sl] = yk[:, 0]
        out[:, 4, sl] = yk[:, 1]
        out[:, 8, sl] = yk[:, 2]
        out[:, 1, sl] = yk[:, 3]
        out[:, 3, sl] = yk[:, 3]
        out[:, 2, sl] = yk[:, 4]
        out[:, 6, sl] = yk[:, 4]
        out[:, 5, sl] = yk[:, 5]
        out[:, 7, sl] = yk[:, 5]
    return out.reshape(x.shape)


# revision 23
# speedup vs baseline: 1.4081x; 1.0231x over previous
"""Trainium2 Bass kernel for per-voxel 3x3 SPD matrix logarithm.

Input  x: (2, 9, 64, 128, 128) fp32, channel c = 3*i+j of symmetric M.
Output Y: same shape, Y = U log(S) U^T per voxel.

Branchless spectral method (fp16 end-to-end on device):
  q = tr(M)/3, D = M - q I, p = sqrt(tr(D^2)/6), r = det(D)/(2 p^3)
  phi = acos(clamp(r))/3 via arctan + sin on ACT
  nodes l1<l2<l3 = q + 2p cos(phi + {-, 0, +}2pi/3), gaps floored at dlo
  divided differences c1, c2 of ln at the nodes (self-consistent: gaps are
  the same f16 values used to build the nodes, so cancellation noise in the
  interpolation coefficients is self-limiting)
  Y = c2 D^2 + e1 D + gam I with e1 = c1 + 2 p cos(phi) c2,
  gam = ln(l1) + (q-l1)(c1 + c2 (q-l2))

fp16 device I/O: host converts input to fp16 (6 unique channels) and expands
the 6 fp16 output channels back to the 9-channel fp32 tensor.

Engines: DVE does fp16 TensorTensor (2x mode) / tensor_scalar (4x mode) bulk,
custom DVE ops for clamp+cube and fp16 reciprocal (BITWISE_NOT seed + NR);
ACT does squares, 1/sqrt (Abs_reciprocal_sqrt), arctan/sin, ln; Pool takes
structural 3-channel products (tau, cross, w3, one output mult).
"""
import math
import numpy as np

import concourse.bacc as bacc
import concourse.tile as tile
import concourse.bass as bass
from concourse import mybir
from concourse.bass_utils import run_bass_kernel_spmd

F32 = mybir.dt.float32
F16 = mybir.dt.float16
OP = mybir.AluOpType
AF = mybir.ActivationFunctionType

B = 2
NV = 64 * 128 * 128
NCORE = 8
VPC = NV // NCORE
P = 128
import os
FD = int(os.environ.get("LOGM_FD", "512"))
NBUF = int(os.environ.get("LOGM_NBUF", "2"))
NBUF_B = int(os.environ.get("LOGM_NBUF_B", "2"))
POOL_A = os.environ.get("LOGM_POOL_A", "0") == "1"
TAU_DVE = os.environ.get("LOGM_TAU_DVE", "1") == "1"
NO_POOL = os.environ.get("LOGM_NO_POOL", "0") == "1"
XBUF = int(os.environ.get("LOGM_XBUF", "4"))
CBUF = int(os.environ.get("LOGM_CBUF", "4"))
AHEAD = int(os.environ.get("LOGM_AHEAD", "2"))
ABUF = int(os.environ.get("LOGM_ABUF", "2"))
SOFF = int(os.environ.get("LOGM_SOFF", "0"))  # 1=q16 2=tt3 4=e1scale on ACT
PMASK = int(os.environ.get("LOGM_PMASK", "23"))  # 1=cross 2=w3 4=vv 8=m2 16=DD2
CPB = VPC // (P * FD)        # chunks per batch
NCHUNK = B * CPB
PLANE = VPC // P

CL = 0.99999988
S3 = math.sqrt(3.0)
PI6 = math.pi / 6.0
DLO = 0.01                   # eigen-gap floor (fp16 consistency scale)

# ---- runtime-registered custom DVE ops ----
from concourse import dve_ops as _dvo
from concourse.dve_spec import (
    Spec as _Spec, Src0 as _S0, Src1 as _S1, C0 as _C0, C1 as _C1, C2 as _C2,
    maxx as _maxx, minn as _minn, lower as _lower, _has_src1 as _hs1,
    Bin as _Bin, AluOp as _AluOp,
)
from concourse.dve_uop import DveOpSpec as _DveOpSpec


def _register_dve(name, spec):
    if name in _dvo._SUB_OPCODE_FOR_NAME:
        return next(op for op in _dvo.OPS if op.name == name)
    op = _dvo.DveOp(name, spec, subdim=False, uops_sha={})
    _dvo.OPS.append(op)
    _dvo.CUSTOM_DVE_SPECS[name] = spec
    row = _dvo._CUSTOM_DVE_ROW_BASE + len(_dvo.OPS) - 1
    assert row < 0x20
    _dvo._SUB_OPCODE_FOR_NAME[name] = row
    for ver in ("v3", "v4"):
        uops = _lower(spec, ver=ver)
        res = _DveOpSpec(name=name, opcode=row, uops=uops, rd1_en=_hs1(spec))
        op.uops_sha[ver] = res.sha(ver)
    return op


# r = clamp(in0 * in1^3 * s0, s1, imm2)
DETC_CLAMP = _register_dve("LOGM_DETC_CLAMP", _Spec(
    body=_minn(_maxx(_S0 * (_S1 * _S1 * _S1) * _C0, _C1), _C2),
    reference=lambda in0, in1, s0, s1, imm2: np.minimum(
        np.maximum(in0.astype(np.float32) * (in1.astype(np.float32) ** 3) * s0, s1), imm2
    ).astype(np.float32),
))

# fp16-capable approximate reciprocal (fp32 datapath: BITWISE_NOT seed + 2 NR)
_not_x = _Bin(_AluOp.BITWISE_NOT, _S0, _S0)
_y0 = _not_x * _C0
_y1 = _y0 * (_C1 - _S0 * _y0)


def _ref_recip_fast(in0, in1, c0, c1, c2):
    not_x = (~in0.astype(np.float32).view(np.int32)).view(np.float32)
    y0 = not_x * c0
    y1 = y0 * (c1 - in0.astype(np.float32) * y0)
    return y1 * (c2 - in0.astype(np.float32) * y1)


RECIP16 = _register_dve("LOGM_RECIP16", _Spec(
    body=_y1 * (_C2 - _S0 * _y1), reference=_ref_recip_fast))
RECIP_CONSTS = {"s0": -0.23549792, "s1": 2.0017324, "imm2": 2.0}

# Pin Arctan to trig_and_small (it genuinely lives there) so arctan->sin needs
# one table set; pin Square to abs_reciprocal_sqrt_and_small (square is in
# every set) so squares ride the ARS load. Both only steer set CHOICE.
from concourse import hw_specs as _hw
import concourse.bacc as _bacc_mod
_orig_gat = _hw.get_activation_tables


def _patched_gat(arch):
    t = _orig_gat(arch)
    for sname, fns in t.items():
        if sname != "trig_and_small":
            fns.discard(mybir.ActivationFunctionType.Arctan)
        if sname != "abs_reciprocal_sqrt_and_small":
            fns.discard(mybir.ActivationFunctionType.Square)
    return t


_hw.get_activation_tables = _patched_gat
_bacc_mod.get_activation_tables = _patched_gat

_CACHE = {}


def _register_const(nc, val):
    t = nc.alloc_sbuf_tensor(f"const-f32-{val}", [128, 1], F32)
    nc.gpsimd.memset(t.ap(), val)
    nc.const_aps.aps[(F32, float(val))] = t.ap()


def build():
    nc = bacc.Bacc("TRN2")
    _register_const(nc, PI6)
    _register_const(nc, PI6 + math.pi / 2.0)
    _register_const(nc, 1.0)
    nc.all_engine_barrier()
    xin = nc.dram_tensor("xin", [B, 6, VPC], F16, kind="ExternalInput")
    yout = nc.dram_tensor("yout", [B, 6, VPC], F16, kind="ExternalOutput")

    V, S = nc.vector, nc.scalar
    G = nc.vector if NO_POOL else nc.gpsimd

    with tile.TileContext(nc) as tc:
        with tc.tile_pool(name="mp", bufs=1) as pool:

            def T2(units, name, dt=F16, bufs=None):
                bufs = NBUF if bufs is None else bufs
                return pool.tile([P, units * FD], dt, name=name, tag=name, bufs=bufs)

            def b3(ap_fd):
                return ap_fd.unsqueeze(1).broadcast_to((P, 3, FD))

            def b2(ap_fd):
                return ap_fd.unsqueeze(1).broadcast_to((P, 2, FD))

            def c3(t):
                return t[:].rearrange("p (c f) -> p c f", c=3)

            def stageA(ci):
                b, j = divmod(ci, CPB)
                t = {}
                xin_t = T2(6, "xin", bufs=(XBUF or NBUF))
                t["xin"] = xin_t
                src1 = bass.AP(xin, b * 6 * VPC + j * FD,
                               [[PLANE, P], [VPC, 3], [1, FD]])
                src2 = bass.AP(xin, b * 6 * VPC + 3 * VPC + j * FD,
                               [[PLANE, P], [VPC, 3], [1, FD]])
                nc.sync.dma_start(xin_t[:, 0:3 * FD].rearrange("p (c f) -> p c f", c=3), src1)
                nc.sync.dma_start(xin_t[:, 3 * FD:6 * FD].rearrange("p (c f) -> p c f", c=3), src2)
                a_ = xin_t[:, 0 * FD:1 * FD]
                d_ = xin_t[:, 1 * FD:2 * FD]
                f_ = xin_t[:, 2 * FD:3 * FD]
                b_ = xin_t[:, 3 * FD:4 * FD]
                c_ = xin_t[:, 4 * FD:5 * FD]
                e_ = xin_t[:, 5 * FD:6 * FD]
                adf = xin_t[:, 0:3 * FD].rearrange("p (c f) -> p c f", c=3)
                bce = xin_t[:, 3 * FD:6 * FD]

                s1 = T2(1, "s1", bufs=ABUF)
                E1 = G if POOL_A else V
                E1.tensor_tensor(s1[:], a_, d_, OP.add)
                E1.tensor_tensor(s1[:], s1[:], f_, OP.add)
                q = T2(1, "q", bufs=(CBUF or NBUF))
                if SOFF & 1:
                    S.activation(q[:], s1[:], AF.Copy, scale=1.0 / 3.0)
                else:
                    V.tensor_scalar(q[:], s1[:], 1.0 / 3.0, None, OP.mult)
                t["q"] = q

                Dt = T2(3, "Dt", bufs=(CBUF or NBUF))
                V.tensor_tensor(c3(Dt), adf, b3(q[:]), OP.subtract)
                t["Dt"] = Dt
                aa = Dt[:, 0:FD]
                dd = Dt[:, FD:2 * FD]

                SQ = T2(6, "SQ", bufs=(CBUF or NBUF))
                S.activation(SQ[:, 0:3 * FD], Dt[:], AF.Square)
                sq_i = S.activation(SQ[:, 3 * FD:6 * FD], bce, AF.Square)
                t["SQ"] = SQ
                t["sq_inst"] = sq_i
                SQb3 = SQ[:, 3 * FD:6 * FD].rearrange("p (c f) -> p c f", c=3)

                stsu = T2(2, "stsu", bufs=(CBUF or NBUF))
                pA = SQ[:].rearrange("p (c f) -> p c f", c=6)
                st2 = stsu[:].rearrange("p (c f) -> p c f", c=2)
                V.tensor_tensor(st2, pA[:, 0:6:3, :], pA[:, 1:6:3, :], OP.add)
                V.tensor_tensor(st2, st2, pA[:, 2:6:3, :], OP.add)
                t["stsu"] = stsu
                p2s6 = T2(1, "p2s6", bufs=(CBUF or NBUF))
                V.scalar_tensor_tensor(p2s6[:], stsu[:, FD:2 * FD], 2.0,
                                       stsu[:, 0:FD], OP.mult, OP.add)
                t["p2s6"] = p2s6

                # det(D): tau_i = Dt_i * SQ_bce_rev_i on Pool
                tau = T2(3, "tau", bufs=ABUF)
                (V if TAU_DVE else G).tensor_tensor(c3(tau), c3(Dt), SQb3[:, ::-1, :], OP.mult)
                dets = T2(1, "dets", bufs=ABUF)
                EA = G if POOL_A else V
                EA.tensor_tensor(dets[:], tau[:, 0:FD], tau[:, FD:2 * FD], OP.add)
                EA.tensor_tensor(dets[:], dets[:], tau[:, 2 * FD:3 * FD], OP.add)
                ad3 = T2(1, "ad3", bufs=ABUF)
                EA.tensor_tensor(ad3[:], aa, dd, OP.mult)
                EA.tensor_tensor(ad3[:], ad3[:], Dt[:, 2 * FD:3 * FD], OP.mult)
                det = T2(1, "det", bufs=ABUF)
                V.tensor_tensor(det[:], ad3[:], dets[:], OP.subtract)

                cross = T2(3, "cross", bufs=(CBUF or NBUF))
                cb_ap = xin_t[:, 3 * FD:5 * FD].rearrange("p (c f) -> p c f", c=2)[:, ::-1, :]
                EC = G if (PMASK & 1) else V
                EC.tensor_tensor(cross[:, 0:2 * FD].rearrange("p (c f) -> p c f", c=2),
                                cb_ap, b2(e_), OP.mult)
                EC.tensor_tensor(cross[:, 2 * FD:3 * FD], b_, c_, OP.mult)
                t["cross"] = cross
                bcep = T2(1, "bcep", bufs=(CBUF or NBUF))
                V.tensor_tensor(bcep[:], cross[:, 2 * FD:3 * FD], e_, OP.mult)
                t["bcep"] = bcep
                det32 = T2(1, "det32", bufs=(CBUF or NBUF))
                V.tensor_scalar(det32[:], bcep[:], 2.0, None, OP.mult)
                V.tensor_tensor(det32[:], det32[:], det[:], OP.add)
                t["det32"] = det32
                return t

            def stageB(ci, t):
                def T2B(units, name, dt=F16):
                    return pool.tile([P, units * FD], dt, name=name, tag=name,
                                     bufs=NBUF_B)
                b, j = divmod(ci, CPB)
                xin_t = t["xin"]; Dt = t["Dt"]; SQ = t["SQ"]
                q = t["q"]; p2s6 = t["p2s6"]; cross = t["cross"]
                stsu = t["stsu"]; det32 = t["det32"]
                Dt3 = c3(Dt)
                bce = xin_t[:, 3 * FD:6 * FD]
                su = stsu[:, FD:2 * FD]

                # --- scalar chain ---
                ip = T2B(1, "ip")
                S.activation(ip[:], p2s6[:], AF.Abs_reciprocal_sqrt, scale=6.0)
                pt16 = T2B(1, "pt16")
                V.tensor_tensor(pt16[:], p2s6[:], ip[:], OP.mult)
                rr = T2B(1, "rr")
                V._custom_dve(DETC_CLAMP, out=rr[:], in0=det32[:], in1=ip[:],
                              s0=108.0, s1=-CL, imm2=CL)
                r2 = T2B(1, "r2")
                S.activation(r2[:], rr[:], AF.Square)
                isq = T2B(1, "isq")
                S.activation(isq[:], r2[:], AF.Abs_reciprocal_sqrt, scale=-1.0, bias=1.0)
                tq = T2B(1, "tq")
                V.tensor_tensor(tq[:], rr[:], isq[:], OP.mult)
                at = T2B(1, "at", dt=F32)
                S.activation(at[:], tq[:], AF.Arctan)
                sfcf = T2B(2, "sfcf")
                S.activation(sfcf[:, 0:FD], at[:], AF.Sin, scale=-1.0 / 3.0, bias=PI6)
                S.activation(sfcf[:, FD:2 * FD], at[:], AF.Sin, scale=-1.0 / 3.0,
                             bias=PI6 + math.pi / 2.0)

                pcps = T2B(2, "pcps")
                V.tensor_tensor(pcps[:].rearrange("p (c f) -> p c f", c=2),
                                b2(pt16[:]), sfcf[:].rearrange("p (c f) -> p c f", c=2),
                                OP.mult)
                ps = pcps[:, 0:FD]; pc = pcps[:, FD:2 * FD]
                tsp = T2B(1, "tsp")
                V.tensor_scalar(tsp[:], ps, S3, None, OP.mult)
                uu = T2B(1, "uu")
                V.tensor_tensor(uu[:], tsp[:], pc, OP.add)

                LD = T2B(6, "LD")  # [l1|l2|l3|d12|d23|d13]
                l1 = LD[:, 0:FD]; l2 = LD[:, FD:2 * FD]; l3 = LD[:, 2 * FD:3 * FD]
                d23 = LD[:, 4 * FD:5 * FD]
                V.tensor_tensor(l1, q[:], uu[:], OP.subtract)
                V.tensor_scalar(LD[:, 3 * FD:4 * FD], ps, 2.0 * S3, DLO, OP.mult, OP.max)
                tt3 = T2B(1, "tt3")
                if SOFF & 2:
                    S.activation(tt3[:], pc, AF.Copy, scale=3.0)
                else:
                    V.tensor_scalar(tt3[:], pc, 3.0, None, OP.mult)
                V.tensor_tensor(d23, tt3[:], tsp[:], OP.subtract)
                V.tensor_scalar(d23, d23, 1.0, DLO, OP.mult, OP.max)
                LD6 = LD[:].rearrange("p (c f) -> p c f", c=6)
                # (l2, d13) = (l1, d12) + (d12, d23)
                V.tensor_tensor(LD6[:, 1:6:4, :], LD6[:, 0:4:3, :], LD6[:, 3:5:1, :], OP.add)
                V.tensor_tensor(l3, l2, d23, OP.add)

                ii = T2B(3, "ii")
                V._custom_dve(RECIP16, out=ii[:], in0=LD[:, 3 * FD:6 * FD], **RECIP_CONSTS)
                lg = T2B(3, "lg")
                S.activation(lg[:], LD[:, 0:3 * FD], AF.Ln)
                gg = T2B(2, "gg")
                V.tensor_tensor(gg[:], lg[:, FD:3 * FD], lg[:, 0:2 * FD], OP.subtract)
                c1f = T2B(2, "c1f")
                V.tensor_tensor(c1f[:], gg[:], ii[:, 0:2 * FD], OP.mult)
                c1 = c1f[:, 0:FD]
                c2 = T2B(1, "c2")
                V.tensor_tensor(c2[:], c1f[:, FD:2 * FD], c1, OP.subtract)
                V.tensor_tensor(c2[:], c2[:], ii[:, 2 * FD:3 * FD], OP.mult)

                # tail: e1 = c1 + 2 pc c2 ; gam = lg1 + uu*(e1 - uu*c2)
                tpc = T2B(1, "tpc")
                V.tensor_tensor(tpc[:], pc, c2[:], OP.mult)
                e1 = T2B(1, "e1")
                if SOFF & 4:
                    S.activation(e1[:], tpc[:], AF.Copy, scale=2.0)
                else:
                    V.tensor_scalar(e1[:], tpc[:], 2.0, None, OP.mult)
                V.tensor_tensor(e1[:], e1[:], c1, OP.add)
                uc2 = T2B(1, "uc2")
                V.tensor_tensor(uc2[:], uu[:], c2[:], OP.mult)
                gam = T2B(1, "gam")
                V.tensor_tensor(gam[:], e1[:], uc2[:], OP.subtract)
                V.tensor_tensor(gam[:], uu[:], gam[:], OP.mult)
                V.tensor_tensor(gam[:], gam[:], lg[:, 0:FD], OP.add)

                # --- outputs ---
                # diag: Yd = c2*(SQ_adf + w3) + e1*Dt + gam
                w3 = T2B(3, "w3")
                SQb3 = SQ[:, 3 * FD:6 * FD].rearrange("p (c f) -> p c f", c=3)
                (G if (PMASK & 2) else V).tensor_tensor(c3(w3), b3(su), SQb3[:, ::-1, :], OP.subtract)
                DD2 = T2B(3, "DD2")
                (G if (PMASK & 16) else V).tensor_tensor(DD2[:], SQ[:, 0:3 * FD], w3[:], OP.add)
                V.tensor_tensor(c3(DD2), c3(DD2), b3(c2[:]), OP.mult)
                m1 = DD2
                u1 = T2B(3, "u1")
                V.tensor_tensor(c3(u1), Dt3[:, ::-1, :], b3(c2[:]), OP.mult)
                (G if (PMASK & 8) else V).tensor_tensor(Dt3, Dt3, b3(e1[:]), OP.mult)
                m2 = Dt
                V.tensor_tensor(m1[:], m1[:], m2[:], OP.add)
                V.tensor_tensor(c3(m1), c3(m1), b3(gam[:]), OP.add)
                Yd = m1

                # offdiag: Yo = bce*(e1 - c2*Dt_rev) + c2*cross
                V.tensor_tensor(c3(u1), b3(e1[:]), c3(u1), OP.subtract)
                V.tensor_tensor(u1[:], bce, u1[:], OP.mult)
                vv = T2B(3, "vv")
                (G if (PMASK & 4) else V).tensor_tensor(c3(vv), c3(cross), b3(c2[:]), OP.mult)
                V.tensor_tensor(u1[:], u1[:], vv[:], OP.add)
                Yo = u1

                dstd = bass.AP(yout, b * 6 * VPC + j * FD,
                               [[PLANE, P], [VPC, 3], [1, FD]])
                dsto = bass.AP(yout, b * 6 * VPC + 3 * VPC + j * FD,
                               [[PLANE, P], [VPC, 3], [1, FD]])
                nc.sync.dma_start(dstd, c3(Yd))
                nc.sync.dma_start(dsto, c3(Yo))

            tiles = {}
            for k in range(min(AHEAD, NCHUNK)):
                tiles[k] = stageA(k)
            for ci in range(NCHUNK):
                if ci + AHEAD < NCHUNK:
                    tiles[ci + AHEAD] = stageA(ci + AHEAD)
                stageB(ci, tiles.pop(ci))
    nc.finalize()
    return nc


def kernel(x):
    x = np.asarray(x)
    xh = x.reshape(B, 9, NV).astype(np.float16)
    sel = [0, 4, 8, 1, 2, 5]  # a d f b c e
    in_maps = []
    for k in range(NCORE):
        shard = np.ascontiguousarray(xh[:, sel, k * VPC:(k + 1) * VPC])
        in_maps.append({"xin": shard})
    if "nc" not in _CACHE:
        _CACHE["nc"] = build()
    res = run_bass_kernel_spmd(_CACHE["nc"], in_maps, core_ids=list(range(NCORE)))
    out = np.empty((B, 9, NV), np.float32)
    for k in range(NCORE):
        sl = slice(k * VPC, (k + 1) * VPC)
        yk = res.results[k]["yout"].astype(np.float32)
        out[:, 0, sl] = yk[:, 0]
        out[:, 4, sl] = yk[:, 1]
        out[:, 8, sl] = yk[:, 2]
        out[:, 1, sl] = yk[:, 3]
        out[:, 3, sl] = yk[:, 3]
        out[:, 2, sl] = yk[:, 4]
        out[:, 6, sl] = yk[:, 4]
        out[:, 5, sl] = yk[:, 5]
        out[:, 7, sl] = yk[:, 5]
    return out.reshape(x.shape)
